# revision 9
# baseline (speedup 1.0000x reference)
"""TP-8 Trainium2 Bass kernel for a LLaDA/Llama transformer block.

Shapes (hardcoded): x [2, 1024, 4096], 32 heads x 128 head_dim,
FF=12288, non-causal attention, RMSNorm + RoPE + SwiGLU.

Sharding (per sharding_hint): tensor-parallel over the 8 cores —
q/k/v/ff sharded on the output-feature axis (4 heads / 1536 ff dims per
core), wo/w_out sharded on the contraction axis.  One fp16 on-device
AllReduce PER BATCH (2 chunks) restores the residual stream after
attention; the final projection partials are summed on the host.

v3 structure (vs the 2.20ms v2):
 - Head-interleaved attention: the q/k projection matmuls of head h+1
   are emitted as filler between the attention chunks of head h, so
   the PE never stalls on the ACT exp latency.
 - wv is loaded once and kept resident across both batches; wqk tiles
   use a 4-deep ring so batch-1 weights prefetch during batch-0's
   attention (keeps model DMA out of the AllReduce window).
 - o-proj residual writes go out on the gpsimd (SWDGE) ring, keeping
   the sync HWDGE ring free of slow-waiting writes: the MLP's x_mid
   loads dispatch as soon as their AllReduce chunk lands instead of
   queueing behind batch-1's writes (v2 lost ~300us to this).
 - AllReduce is split into 2 chunks per batch so the first half of
   x_mid is available earlier.
 - norm1 on host (pre-normalized xnT_h input), fp16 residual + output,
   norm2 merged into the MLP block, fast DVE reciprocal.
"""

from contextlib import ExitStack

import numpy as np

import concourse.mybir as mybir
import concourse.tile as tile
from concourse import bacc
from concourse.bass_utils import run_bass_kernel_spmd

F32 = mybir.dt.float32
F16 = mybir.dt.float16
AF = mybir.ActivationFunctionType
ALU = mybir.AluOpType

N_CORES = 8
P = 128
B, T, D, FF = 2, 1024, 4096, 12288
M = B * T            # 2048 tokens
H = 128              # head dim
HALF = 64
QC = D // N_CORES    # 512 per-core q/k/v features (4 heads)
NH = QC // H         # 4 heads per core
FC = FF // N_CORES   # 1536 per-core ff features
NKP = D // P         # 32 K-tiles over D
NFT = FC // P        # 12 M-tiles over per-core FF
NDT = D // P         # 32 D-tiles
NST = T // P         # 8 sequence tiles per batch
EPS = 1e-05
LA = 2               # attention pipeline lookahead (512-col chunks)


def _cs(ch):
    return slice(ch * 512, (ch + 1) * 512)


def _build():
    nc = bacc.Bacc("TRN2", target_bir_lowering=False, num_devices=N_CORES)

    xT_h = nc.declare_dram_parameter("xT_h", [D, M], F16, isOutput=False)
    xnT_h = nc.declare_dram_parameter("xnT_h", [D, M], F16, isOutput=False)
    css = nc.declare_dram_parameter("css", [2, P, M], F16, isOutput=False)
    wq_t = nc.declare_dram_parameter("wq_t", [NH, P, NKP, P], F16, isOutput=False)
    wk_t = nc.declare_dram_parameter("wk_t", [NH, P, NKP, P], F16, isOutput=False)
    wv_n = nc.declare_dram_parameter("wv_n", [D, QC], F16, isOutput=False)
    wo_t = nc.declare_dram_parameter("wo_t", [NH, P, NDT, P], F16, isOutput=False)
    wf_t = nc.declare_dram_parameter("wf_t", [NFT, P, NKP, P], F16, isOutput=False)
    wu_t = nc.declare_dram_parameter("wu_t", [NFT, P, NKP, P], F16, isOutput=False)
    wout_t = nc.declare_dram_parameter("wout_t", [NDT, P, NFT, P], F16, isOutput=False)
    y = nc.declare_dram_parameter("y", [D, M], F16, isOutput=True)

    with tile.TileContext(nc) as tc:
        _emit(nc, tc, xT_h, xnT_h, css, wq_t, wk_t, wv_n, wo_t, wf_t, wu_t,
              wout_t, y)
    nc.compile()
    return nc


def _emit(nc, tc, xT_h, xnT_h, css, wq_t, wk_t, wv_n, wo_t, wf_t, wu_t,
          wout_t, y):
    with ExitStack() as top:
        dram_pool = top.enter_context(tc.tile_pool(name="dram", bufs=1, space="DRAM"))
        const = top.enter_context(tc.tile_pool(name="const", bufs=1))

        cc_in = [dram_pool.tile([D, T], F16, name=f"cc_in_{b}") for b in range(B)]
        cc_half = [
            [
                dram_pool.tile([D // 2, T], F16, addr_space="Shared",
                               name=f"cc_out_{b}_{k}")
                for k in range(2)
            ]
            for b in range(B)
        ]

        ones_h = const.tile([P, P], F16)
        nc.vector.memset(ones_h[:], 1.0)
        cc_sb = const.tile([P, M], F16)
        ss_sb = const.tile([P, M], F16)
        nc.sync.dma_start(out=cc_sb[:], in_=css[0])
        nc.sync.dma_start(out=ss_sb[:], in_=css[1])
        eps_sb = const.tile([P, 1], F32)
        nc.vector.memset(eps_sb[:], EPS)
        bcast2 = [const.tile([P, T], F16, name=f"bcast2_{b}") for b in range(B)]

        # ---------- attention half: qkv + rope + attn + o-proj + AR ----------
        with ExitStack() as asec:
            wvp = asec.enter_context(tc.tile_pool(name="wv_res", bufs=1))
            wv_sb = []
            for b in range(B):
                _attn_batch(nc, tc, b, wv_sb, wvp, xnT_h, xT_h, wq_t, wk_t,
                            wv_n, wo_t, ones_h, cc_sb, ss_sb, cc_in, cc_half)

        # ---------------- norm2 + SwiGLU MLP, per batch ----------------
        for b in range(B):
            _mlp_batch(nc, tc, b, cc_half, ones_h, eps_sb, bcast2, wf_t, wu_t,
                       wout_t, y)


def _attn_batch(nc, tc, b, wv_sb, wvp, xnT_h, xT_h, wq_t, wk_t, wv_n, wo_t,
                ones_h, cc_sb, ss_sb, cc_in, cc_half):
    bs = slice(b * T, (b + 1) * T)
    with ExitStack() as bph:
        bp = bph.enter_context(tc.tile_pool(name=f"bat_{b}", bufs=1))
        afp = bph.enter_context(tc.tile_pool(name=f"attnf_{b}", bufs=1))
        attnf = []
        inner = bph.enter_context(ExitStack())
        xp = inner.enter_context(tc.tile_pool(name=f"xn_{b}", bufs=1))
        sp = inner.enter_context(tc.tile_pool(name=f"qkv_{b}", bufs=1))

        xn = []
        for kp in range(NKP):
            xnk = xp.tile([P, T], F16, tag=f"xn{kp}", name=f"xn_{b}_{kp}")
            nc.sync.dma_start(out=xnk[:], in_=xnT_h[kp * P : (kp + 1) * P, bs])
            xn.append(xnk)

        # v projection (token-major); wv loads once, resident across batches
        v_sb = []
        with ExitStack() as vph:
            vpp = vph.enter_context(
                tc.tile_pool(name=f"v_ps_{b}", bufs=1, space="PSUM")
            )
            ps_v = [
                vpp.tile([P, QC], F32, tag=f"vps{st}", name=f"psv_{b}_{st}")
                for st in range(NST)
            ]
            for kp in range(NKP):
                if b == 0:
                    wvk = wvp.tile([P, QC], F16, tag=f"wv{kp}", name=f"wv_{kp}")
                    nc.sync.dma_start(
                        out=wvk[:], in_=wv_n[kp * P : (kp + 1) * P, :]
                    )
                    wv_sb.append(wvk)
                for st in range(NST):
                    nc.tensor.matmul(
                        ps_v[st][:],
                        xn[kp][:, st * P : (st + 1) * P],
                        wv_sb[kp][:],
                        start=(kp == 0),
                        stop=(kp == NKP - 1),
                    )
            for st in range(NST):
                vt = xp.tile([P, QC], F16, tag=f"v{st}", name=f"v_{b}_{st}")
                nc.scalar.copy(vt[:], ps_v[st][:])
                v_sb.append(vt)

        # q/k projection thunks: 8 filler-sized pieces per (which, head),
        # rope eviction inside the last piece.
        qpp = inner.enter_context(tc.tile_pool(name=f"qk_ps_{b}", bufs=1, space="PSUM"))
        qf, kf = [None] * NH, [None] * NH

        def make_proj_thunks(which, wsrc, h, dst, idx):
            wt = sp.tile([P, NKP, P], F16, tag="wqk", bufs=4,
                         name=f"w{which}_{b}_{h}")
            nc.sync.dma_start(out=wt[:], in_=wsrc[h])
            state = {}

            def piece(i):
                def run():
                    if i == 0:
                        state["ps"] = qpp.tile(
                            [P, T], F32, tag="qk_ps", bufs=1,
                            name=f"ps{which}_{b}_{h}",
                        )
                    ps = state["ps"]
                    for kp in range(i * 4, i * 4 + 4):
                        for ch in range(2):
                            nc.tensor.matmul(
                                ps[:, _cs(ch)],
                                wt[:, kp, :],
                                xn[kp][:, _cs(ch)],
                                start=(kp == 0),
                                stop=(kp == NKP - 1),
                            )
                    if i == 7:
                        ps = state["ps"]
                        main = sp.tile([P, T], F16, tag="rmain", bufs=2,
                                       name=f"rm_{which}_{b}_{h}")
                        nc.vector.scalar_tensor_tensor(
                            main[:], ps[:], 1.0, cc_sb[:, bs],
                            ALU.mult, ALU.mult,
                        )
                        rot = sp.tile([P, T], F16, tag="rrot", bufs=2,
                                      name=f"rr_{which}_{b}_{h}")
                        nc.vector.scalar_tensor_tensor(
                            rot[:HALF], ps[HALF:], -1.0,
                            ss_sb[:HALF, bs], ALU.mult, ALU.mult,
                        )
                        nc.vector.scalar_tensor_tensor(
                            rot[HALF:], ps[:HALF], 1.0,
                            ss_sb[HALF:, bs], ALU.mult, ALU.mult,
                        )
                        out = xp.tile([P, T], F16, tag=f"{which}f{h}",
                                      name=f"{which}f_{b}_{h}")
                        nc.vector.tensor_add(out[:], main[:], rot[:])
                        dst[idx] = out
                return run

            return [piece(i) for i in range(8)]

        def head_thunks(h):
            return (make_proj_thunks("q", wq_t, h, qf, h)
                    + make_proj_thunks("k", wk_t, h, kf, h))

        # prologue: head 0's projections run un-interleaved
        for t in head_thunks(0):
            t()

        # attention per head, chunk-pipelined, filler = head h+1 projections
        ap_ = inner.enter_context(tc.tile_pool(name=f"att_{b}", bufs=1))
        app = inner.enter_context(tc.tile_pool(name=f"att_ps_{b}", bufs=1, space="PSUM"))
        for h in range(NH):
            filler = head_thunks(h + 1) if h + 1 < NH else []
            den = [
                app.tile([P, 512], F32, tag=f"den{ch}", name=f"den_{b}_{h}_{ch}")
                for ch in range(2)
            ]
            at = [
                app.tile([P, 512], F32, tag=f"at{ch}", name=f"at_{b}_{h}_{ch}")
                for ch in range(2)
            ]

            def emit_lg(j, h=h):
                st, ch = divmod(j, 2)
                lg = app.tile([P, 512], F32, tag="lg", bufs=2,
                              name=f"lg_{b}_{h}_{j}")
                nc.tensor.matmul(
                    lg[:],
                    kf[h][:, st * P : (st + 1) * P],
                    qf[h][:, _cs(ch)],
                    start=True,
                    stop=True,
                )
                pr = ap_.tile([P, 512], F16, tag="pr", bufs=6,
                              name=f"pr_{b}_{h}_{j}")
                nc.scalar.activation(pr[:], lg[:], AF.Exp)
                return pr

            prs = [None] * 16
            for j in range(LA):
                prs[j] = emit_lg(j)
            for j in range(16):
                if j + LA < 16:
                    prs[j + LA] = emit_lg(j + LA)
                st, ch = divmod(j, 2)
                pr = prs[j]
                nc.tensor.matmul(
                    den[ch][:], ones_h[:], pr[:],
                    start=(st == 0), stop=(st == NST - 1),
                )
                nc.tensor.matmul(
                    at[ch][:],
                    v_sb[st][:, h * H : (h + 1) * H],
                    pr[:],
                    start=(st == 0), stop=(st == NST - 1),
                )
                prs[j] = None
                if filler:
                    filler.pop(0)()
            for t in filler:
                t()
            af = afp.tile([P, T], F16, tag=f"af{h}", name=f"af_{b}_{h}")
            for ch in range(2):
                rec = ap_.tile([P, 512], F32, tag="rec", bufs=4,
                               name=f"rec_{b}_{h}_{ch}")
                nc.vector.reciprocal_approx_fast(out=rec[:], in_=den[ch][:])
                nc.vector.scalar_tensor_tensor(
                    af[:, _cs(ch)], at[ch][:], 1.0, rec[:],
                    ALU.mult, ALU.mult,
                )
            attnf.append(af)

        inner.close()  # free xn / wqk / attention pools before o-proj

        # o-projection partial + residual; writes on the SWDGE ring so the
        # sync ring stays free; 2 AllReduce chunks per batch
        with ExitStack() as ph:
            osp = ph.enter_context(tc.tile_pool(name=f"op_{b}", bufs=1))
            pp = ph.enter_context(
                tc.tile_pool(name=f"op_ps_{b}", bufs=1, space="PSUM")
            )
            wo_sb = []
            for h in range(NH):
                wt = osp.tile([P, NDT, P], F16, tag=f"wo{h}", name=f"wo_{b}_{h}")
                nc.sync.dma_start(out=wt[:], in_=wo_t[h])
                wo_sb.append(wt)
            for dt in range(NDT):
                ps = pp.tile([P, T], F32, tag="o_ps", bufs=2, name=f"pso_{b}_{dt}")
                for h in range(NH):
                    for ch in range(T // 512):
                        nc.tensor.matmul(
                            ps[:, _cs(ch)],
                            wo_sb[h][:, dt, :],
                            attnf[h][:, _cs(ch)],
                            start=(h == 0),
                            stop=(h == NH - 1),
                        )
                xt = osp.tile([P, T], F16, tag="xs3", bufs=3, name=f"xo_{b}_{dt}")
                nc.sync.dma_start(
                    out=xt[:], in_=xT_h[dt * P : (dt + 1) * P, bs]
                )
                osb = osp.tile([P, T], F16, tag="osb", bufs=3, name=f"osb_{b}_{dt}")
                nc.vector.scalar_tensor_tensor(
                    osb[:], xt[:], 1.0 / N_CORES, ps[:], ALU.mult, ALU.add,
                )
                nc.gpsimd.dma_start(
                    out=cc_in[b][dt * P : (dt + 1) * P, :], in_=osb[:]
                )
                if dt in (NDT // 2 - 1, NDT - 1):
                    k = dt // (NDT // 2)
                    rows = slice(k * (D // 2), (dt + 1) * P)
                    nc.gpsimd.collective_compute(
                        "AllReduce",
                        ALU.add,
                        replica_groups=[list(range(N_CORES))],
                        ins=[cc_in[b][rows, :]],
                        outs=[cc_half[b][k][:, :]],
                    )


def _mlp_batch(nc, tc, b, cc_half, ones_h, eps_sb, bcast2, wf_t, wu_t, wout_t, y):
    bs = slice(b * T, (b + 1) * T)
    with ExitStack() as bph:
        bp = bph.enter_context(tc.tile_pool(name=f"mlpb_{b}", bufs=1))
        stp = bph.enter_context(tc.tile_pool(name=f"mstat_{b}", bufs=1))
        spp = bph.enter_context(
            tc.tile_pool(name=f"mstat_ps_{b}", bufs=1, space="PSUM")
        )
        # x_mid tiles: loaded once, reused by stats, ff/up matmuls and the
        # wout residual.
        xmh = []
        ms_ps = spp.tile([P, T], F32, name=f"ms_{b}")
        for kp in range(NKP):
            xk = bp.tile([P, T], F16, tag=f"xm{kp}", name=f"xmh_{b}_{kp}")
            half, loc = divmod(kp, NKP // 2)
            nc.sync.dma_start(
                out=xk[:], in_=cc_half[b][half][loc * P : (loc + 1) * P, :]
            )
            xmh.append(xk)
            sq = stp.tile([P, T], F16, tag="sq", bufs=3, name=f"sq_{b}_{kp}")
            if kp % 2 == 0:
                nc.scalar.activation(sq[:], xk[:], AF.Square)
            else:
                nc.vector.tensor_mul(sq[:], xk[:], xk[:])
            for ch in range(2):
                nc.tensor.matmul(
                    ms_ps[:, _cs(ch)], ones_h[:], sq[:, _cs(ch)],
                    start=(kp == 0), stop=(kp == NKP - 1),
                )
        lnt = stp.tile([P, T], F32, name=f"lnt_{b}")
        nc.scalar.activation(
            lnt[:], ms_ps[:], AF.Ln, bias=eps_sb[:], scale=1.0 / D
        )
        nc.scalar.activation(bcast2[b][:], lnt[:], AF.Exp, scale=-0.5)

        hsb = []
        with ExitStack() as ph:
            sp = ph.enter_context(tc.tile_pool(name=f"mlp_{b}", bufs=1))
            pp = ph.enter_context(
                tc.tile_pool(name=f"mlp_ps_{b}", bufs=1, space="PSUM")
            )
            ffs = []
            for m in range(NFT):
                for which, wsrc in (("f", wf_t), ("u", wu_t)):
                    wt = sp.tile([P, NKP, P], F16, tag="wffu", bufs=3,
                                 name=f"w{which}_{b}_{m}")
                    nc.sync.dma_start(out=wt[:], in_=wsrc[m])
                    ps = pp.tile([P, T], F32, tag="ps_fu", bufs=2,
                                 name=f"ps{which}_{b}_{m}")
                    for kp in range(NKP):
                        for ch in range(2):
                            nc.tensor.matmul(
                                ps[:, _cs(ch)],
                                wt[:, kp, :],
                                xmh[kp][:, _cs(ch)],
                                start=(kp == 0),
                                stop=(kp == NKP - 1),
                            )
                    # fold the norm2 scale into the eviction
                    nt = sp.tile([P, T], F16, tag=f"nrm_{which}", bufs=3,
                                 name=f"nt{which}_{b}_{m}")
                    nc.vector.scalar_tensor_tensor(
                        nt[:], ps[:], 1.0, bcast2[b][:], ALU.mult, ALU.mult,
                    )
                    if which == "f":
                        ft = sp.tile([P, T], F16, tag="ffs", bufs=3,
                                     name=f"ff_{b}_{m}")
                        nc.scalar.activation(ft[:], nt[:], AF.Silu)
                        ffs.append(ft)
                    else:
                        ht = bp.tile([P, T], F16, tag=f"h{m}", name=f"h_{b}_{m}")
                        nc.vector.tensor_mul(ht[:], nt[:], ffs[m][:])
                        hsb.append(ht)

        # w_out projection + residual, partial fp16 output
        with ExitStack() as ph:
            sp = ph.enter_context(tc.tile_pool(name=f"wo2_{b}", bufs=1))
            pp = ph.enter_context(
                tc.tile_pool(name=f"wo2_ps_{b}", bufs=1, space="PSUM")
            )
            for dt in range(NDT):
                wt = sp.tile([P, NFT, P], F16, tag="wot", bufs=3,
                             name=f"wot_{b}_{dt}")
                nc.sync.dma_start(out=wt[:], in_=wout_t[dt])
                ps = pp.tile([P, T], F32, tag="ps_o2", bufs=2,
                             name=f"pso2_{b}_{dt}")
                for m in range(NFT):
                    for ch in range(2):
                        nc.tensor.matmul(
                            ps[:, _cs(ch)],
                            wt[:, m, :],
                            hsb[m][:, _cs(ch)],
                            start=(m == 0),
                            stop=(m == NFT - 1),
                        )
                ysb = sp.tile([P, T], F16, tag="ysb", bufs=3,
                              name=f"ysb_{b}_{dt}")
                nc.vector.scalar_tensor_tensor(
                    ysb[:], xmh[dt][:], 1.0 / N_CORES, ps[:],
                    ALU.mult, ALU.add,
                )
                nc.sync.dma_start(out=y[dt * P : (dt + 1) * P, bs], in_=ysb[:])


_NC_CACHE = {}


def _get_nc():
    if "nc" not in _NC_CACHE:
        _NC_CACHE["nc"] = _build()
    return _NC_CACHE["nc"]


def _host_prep(x, sin, cos, attn_norm_w, ff_norm_w, wq, wk, wv, wo, w_ff, w_up, w_out):
    f16 = np.float16
    x2 = np.asarray(x, np.float32).reshape(M, D)
    xT = np.ascontiguousarray(x2.T)
    rs1 = 1.0 / np.sqrt((xT * xT).mean(0) + EPS)        # [M] norm1 on host
    xnT = xT * rs1[None, :]

    sinT = np.asarray(sin, np.float32).reshape(M, HALF).T
    cosT = np.asarray(cos, np.float32).reshape(M, HALF).T
    cc = np.concatenate([cosT, cosT], axis=0)
    ss = np.concatenate([sinT, sinT], axis=0)
    css = np.stack([cc, ss]).astype(f16)

    anw = np.asarray(attn_norm_w, np.float32)[:, None]
    fnw = np.asarray(ff_norm_w, np.float32)[:, None]
    wqn = (anw * np.asarray(wq, np.float32)) * (H ** -0.5)
    wkn = anw * np.asarray(wk, np.float32)
    wvn = anw * np.asarray(wv, np.float32)
    wfn = fnw * np.asarray(w_ff, np.float32)
    wun = fnw * np.asarray(w_up, np.float32)
    wo = np.asarray(wo, np.float32)
    w_out = np.asarray(w_out, np.float32)

    def mtile(w):
        # [K, F] -> [F/P, P, K/P, P] with [m, p, kp, j] = w[kp*P+p, m*P+j]
        K, F = w.shape
        return np.ascontiguousarray(
            w.reshape(K // P, P, F // P, P).transpose(2, 1, 0, 3)
        )

    in_maps = []
    for c in range(N_CORES):
        qs = slice(c * QC, (c + 1) * QC)
        fs = slice(c * FC, (c + 1) * FC)
        in_maps.append(
            {
                "xT_h": xT.astype(f16),
                "xnT_h": xnT.astype(f16),
                "css": css,
                "wq_t": mtile(wqn[:, qs]).astype(f16),
                "wk_t": mtile(wkn[:, qs]).astype(f16),
                "wv_n": wvn[:, qs].astype(f16),
                # [h, p, dt, j] = wo[c*QC + h*P + p, dt*P + j]
                "wo_t": np.ascontiguousarray(
                    wo[qs, :].reshape(NH, P, NDT, P)
                ).astype(f16),
                "wf_t": mtile(wfn[:, fs]).astype(f16),
                "wu_t": mtile(wun[:, fs]).astype(f16),
                "wout_t": mtile(w_out[fs, :]).astype(f16),
            }
        )
    return in_maps


def kernel(**inputs) -> np.ndarray:
    nc = _get_nc()
    in_maps = _host_prep(**inputs)
    res = run_bass_kernel_spmd(
        nc, in_maps, core_ids=list(range(N_CORES)), trace=False
    )
    acc = res.results[0]["y"].astype(np.float64)
    for c in range(1, N_CORES):
        acc += res.results[c]["y"]
    return np.ascontiguousarray(acc.T).astype(np.float32).reshape(B, T, D)


# revision 17
# speedup vs baseline: 1.0229x; 1.0229x over previous
"""TP-8 Trainium2 Bass kernel for a LLaDA/Llama transformer block.

Shapes (hardcoded): x [2, 1024, 4096], 32 heads x 128 head_dim,
FF=12288, non-causal attention, RMSNorm + RoPE + SwiGLU.

Sharding (per sharding_hint): tensor-parallel over the 8 cores —
q/k/v/ff sharded on the output-feature axis (4 heads / 1536 ff dims per
core), wo/w_out sharded on the contraction axis.  One fp16 on-device
AllReduce PER BATCH (2 chunks) restores the residual stream after
attention; the final projection partials are summed on the host.

v3 structure (vs the 2.20ms v2):
 - Head-interleaved attention: the q/k projection matmuls of head h+1
   are emitted as filler between the attention chunks of head h, so
   the PE never stalls on the ACT exp latency.
 - wv is loaded once and kept resident across both batches; wqk tiles
   use a 4-deep ring so batch-1 weights prefetch during batch-0's
   attention (keeps model DMA out of the AllReduce window).
 - o-proj residual writes go out on the gpsimd (SWDGE) ring, keeping
   the sync HWDGE ring free of slow-waiting writes: the MLP's x_mid
   loads dispatch as soon as their AllReduce chunk lands instead of
   queueing behind batch-1's writes (v2 lost ~300us to this).
 - AllReduce is split into 2 chunks per batch so the first half of
   x_mid is available earlier.
 - norm1 on host (pre-normalized xnT_h input), fp16 residual + output,
   norm2 merged into the MLP block, fast DVE reciprocal.
"""

from contextlib import ExitStack

import numpy as np

import concourse.mybir as mybir
import concourse.tile as tile
from concourse import bacc
from concourse.bass_utils import run_bass_kernel_spmd

F32 = mybir.dt.float32
F16 = mybir.dt.float16
AF = mybir.ActivationFunctionType
ALU = mybir.AluOpType

N_CORES = 8
P = 128
B, T, D, FF = 2, 1024, 4096, 12288
M = B * T            # 2048 tokens
H = 128              # head dim
HALF = 64
QC = D // N_CORES    # 512 per-core q/k/v features (4 heads)
NH = QC // H         # 4 heads per core
FC = FF // N_CORES   # 1536 per-core ff features
NKP = D // P         # 32 K-tiles over D
NFT = FC // P        # 12 M-tiles over per-core FF
NDT = D // P         # 32 D-tiles
NST = T // P         # 8 sequence tiles per batch
EPS = 1e-05
LA = 2               # attention pipeline lookahead (512-col chunks)


def _cs(ch):
    return slice(ch * 512, (ch + 1) * 512)


def _build():
    nc = bacc.Bacc("TRN2", target_bir_lowering=False, num_devices=N_CORES)

    xT_h = nc.declare_dram_parameter("xT_h", [D, M], F16, isOutput=False)
    xnT_h = nc.declare_dram_parameter("xnT_h", [D, M], F16, isOutput=False)
    css = nc.declare_dram_parameter("css", [2, P, M], F16, isOutput=False)
    wq_t = nc.declare_dram_parameter("wq_t", [NH, P, NKP, P], F16, isOutput=False)
    wk_t = nc.declare_dram_parameter("wk_t", [NH, P, NKP, P], F16, isOutput=False)
    wv_n = nc.declare_dram_parameter("wv_n", [D, QC], F16, isOutput=False)
    wo_t = nc.declare_dram_parameter("wo_t", [NH, P, NDT, P], F16, isOutput=False)
    wf_t = nc.declare_dram_parameter("wf_t", [NFT, P, NKP, P], F16, isOutput=False)
    wu_t = nc.declare_dram_parameter("wu_t", [NFT, P, NKP, P], F16, isOutput=False)
    wout_t = nc.declare_dram_parameter("wout_t", [NDT, P, NFT, P], F16, isOutput=False)
    y = nc.declare_dram_parameter("y", [D, M], F16, isOutput=True)

    with tile.TileContext(nc) as tc:
        _emit(nc, tc, xT_h, xnT_h, css, wq_t, wk_t, wv_n, wo_t, wf_t, wu_t,
              wout_t, y)
    nc.compile()
    return nc


def _emit(nc, tc, xT_h, xnT_h, css, wq_t, wk_t, wv_n, wo_t, wf_t, wu_t,
          wout_t, y):
    with ExitStack() as top:
        dram_pool = top.enter_context(tc.tile_pool(name="dram", bufs=1, space="DRAM"))
        const = top.enter_context(tc.tile_pool(name="const", bufs=1))

        cc_in = [dram_pool.tile([D, T], F16, name=f"cc_in_{b}") for b in range(B)]
        cc_half = [
            [
                dram_pool.tile([D // 2, T], F16, addr_space="Shared",
                               name=f"cc_out_{b}_{k}")
                for k in range(2)
            ]
            for b in range(B)
        ]

        ones_h = const.tile([P, P], F16)
        nc.vector.memset(ones_h[:], 1.0)
        cc_sb = const.tile([P, M], F16)
        ss_sb = const.tile([P, M], F16)
        nc.sync.dma_start(out=cc_sb[:], in_=css[0])
        nc.sync.dma_start(out=ss_sb[:], in_=css[1])
        eps_sb = const.tile([P, 1], F32)
        nc.vector.memset(eps_sb[:], EPS)
        bcast2 = [const.tile([P, T], F16, name=f"bcast2_{b}") for b in range(B)]

        # ---------- attention half: qkv + rope + attn + o-proj + AR ----------
        with ExitStack() as asec:
            wvp = asec.enter_context(tc.tile_pool(name="wv_res", bufs=1))
            wv_sb = []
            for b in range(B):
                _attn_batch(nc, tc, b, wv_sb, wvp, xnT_h, xT_h, wq_t, wk_t,
                            wv_n, wo_t, ones_h, cc_sb, ss_sb, cc_in, cc_half)

        # ---------------- norm2 + SwiGLU MLP, per batch ----------------
        for b in range(B):
            _mlp_batch(nc, tc, b, xT_h, cc_half, ones_h, eps_sb, bcast2,
                       wf_t, wu_t, wout_t, y)


def _attn_batch(nc, tc, b, wv_sb, wvp, xnT_h, xT_h, wq_t, wk_t, wv_n, wo_t,
                ones_h, cc_sb, ss_sb, cc_in, cc_half):
    bs = slice(b * T, (b + 1) * T)
    with ExitStack() as bph:
        bp = bph.enter_context(tc.tile_pool(name=f"bat_{b}", bufs=1))
        afp = bph.enter_context(tc.tile_pool(name=f"attnf_{b}", bufs=1))
        attnf = []
        inner = bph.enter_context(ExitStack())
        xp = inner.enter_context(tc.tile_pool(name=f"xn_{b}", bufs=1))
        sp = inner.enter_context(tc.tile_pool(name=f"qkv_{b}", bufs=1))

        # v projection (token-major); wv loads once, resident across batches;
        # xn and wv DMAs interleave so the first matmul starts immediately
        xn = []
        v_sb = []
        with ExitStack() as vph:
            vpp = vph.enter_context(
                tc.tile_pool(name=f"v_ps_{b}", bufs=1, space="PSUM")
            )
            ps_v = [
                vpp.tile([P, QC], F32, tag=f"vps{st}", name=f"psv_{b}_{st}")
                for st in range(NST)
            ]
            for kp in range(NKP):
                xnk = xp.tile([P, T], F16, tag=f"xn{kp}", name=f"xn_{b}_{kp}")
                nc.sync.dma_start(
                    out=xnk[:], in_=xnT_h[kp * P : (kp + 1) * P, bs]
                )
                xn.append(xnk)
                if b == 0:
                    wvk = wvp.tile([P, QC], F16, tag=f"wv{kp}", name=f"wv_{kp}")
                    nc.sync.dma_start(
                        out=wvk[:], in_=wv_n[kp * P : (kp + 1) * P, :]
                    )
                    wv_sb.append(wvk)
                for st in range(NST):
                    nc.tensor.matmul(
                        ps_v[st][:],
                        xn[kp][:, st * P : (st + 1) * P],
                        wv_sb[kp][:],
                        start=(kp == 0),
                        stop=(kp == NKP - 1),
                    )
            for st in range(NST):
                vt = xp.tile([P, QC], F16, tag=f"v{st}", name=f"v_{b}_{st}")
                nc.scalar.copy(vt[:], ps_v[st][:])
                v_sb.append(vt)

        # q/k projection thunks: 8 filler-sized pieces per (which, head),
        # rope eviction inside the last piece.
        qpp = inner.enter_context(tc.tile_pool(name=f"qk_ps_{b}", bufs=1, space="PSUM"))
        qf, kf = [None] * NH, [None] * NH

        # weight prefetch: q/k weights for head h load ~2 heads ahead of use
        wts = {}

        def ensure_w(h):
            if h >= NH or ("q", h) in wts:
                return
            for which, wsrc in (("q", wq_t), ("k", wk_t)):
                wt = sp.tile([P, NKP, P], F16, tag="wqk", bufs=4,
                             name=f"w{which}_{b}_{h}")
                nc.sync.dma_start(out=wt[:], in_=wsrc[h])
                wts[(which, h)] = wt

        def make_proj_thunks(which, h, dst, idx):
            wt = wts[(which, h)]
            state = {}

            def piece(i):
                def run():
                    if i == 0:
                        state["ps"] = qpp.tile(
                            [P, T], F32, tag="qk_ps", bufs=1,
                            name=f"ps{which}_{b}_{h}",
                        )
                    ps = state["ps"]
                    for kp in range(i * 4, i * 4 + 4):
                        for ch in range(2):
                            nc.tensor.matmul(
                                ps[:, _cs(ch)],
                                wt[:, kp, :],
                                xn[kp][:, _cs(ch)],
                                start=(kp == 0),
                                stop=(kp == NKP - 1),
                            )
                    if i == 7:
                        ps = state["ps"]
                        main = sp.tile([P, T], F16, tag="rmain", bufs=2,
                                       name=f"rm_{which}_{b}_{h}")
                        nc.vector.scalar_tensor_tensor(
                            main[:], ps[:], 1.0, cc_sb[:, bs],
                            ALU.mult, ALU.mult,
                        )
                        rot = sp.tile([P, T], F16, tag="rrot", bufs=2,
                                      name=f"rr_{which}_{b}_{h}")
                        nc.vector.scalar_tensor_tensor(
                            rot[:HALF], ps[HALF:], -1.0,
                            ss_sb[:HALF, bs], ALU.mult, ALU.mult,
                        )
                        nc.vector.scalar_tensor_tensor(
                            rot[HALF:], ps[:HALF], 1.0,
                            ss_sb[HALF:, bs], ALU.mult, ALU.mult,
                        )
                        out = xp.tile([P, T], F16, tag=f"{which}f{h}",
                                      name=f"{which}f_{b}_{h}")
                        nc.vector.tensor_add(out[:], main[:], rot[:])
                        dst[idx] = out
                return run

            return [piece(i) for i in range(8)]

        def head_thunks(h):
            return (make_proj_thunks("q", h, qf, h)
                    + make_proj_thunks("k", h, kf, h))

        # prologue: head 0's projections run un-interleaved
        ensure_w(0)
        ensure_w(1)
        for t in head_thunks(0):
            t()

        # attention per head, chunk-pipelined, filler = head h+1 projections
        ap_ = inner.enter_context(tc.tile_pool(name=f"att_{b}", bufs=1))
        app = inner.enter_context(tc.tile_pool(name=f"att_ps_{b}", bufs=1, space="PSUM"))
        for h in range(NH):
            ensure_w(h + 2)
            filler = head_thunks(h + 1) if h + 1 < NH else []
            den = [
                app.tile([P, 512], F32, tag=f"den{ch}", name=f"den_{b}_{h}_{ch}")
                for ch in range(2)
            ]
            at = [
                app.tile([P, 512], F32, tag=f"at{ch}", name=f"at_{b}_{h}_{ch}")
                for ch in range(2)
            ]

            def emit_lg(j, h=h):
                st, ch = divmod(j, 2)
                lg = app.tile([P, 512], F32, tag="lg", bufs=2,
                              name=f"lg_{b}_{h}_{j}")
                nc.tensor.matmul(
                    lg[:],
                    kf[h][:, st * P : (st + 1) * P],
                    qf[h][:, _cs(ch)],
                    start=True,
                    stop=True,
                )
                pr = ap_.tile([P, 512], F16, tag="pr", bufs=6,
                              name=f"pr_{b}_{h}_{j}")
                nc.scalar.activation(pr[:], lg[:], AF.Exp)
                return pr

            prs = [None] * 16
            for j in range(LA):
                prs[j] = emit_lg(j)
            for j in range(16):
                if j + LA < 16:
                    prs[j + LA] = emit_lg(j + LA)
                st, ch = divmod(j, 2)
                pr = prs[j]
                nc.tensor.matmul(
                    den[ch][:], ones_h[:], pr[:],
                    start=(st == 0), stop=(st == NST - 1),
                )
                nc.tensor.matmul(
                    at[ch][:],
                    v_sb[st][:, h * H : (h + 1) * H],
                    pr[:],
                    start=(st == 0), stop=(st == NST - 1),
                )
                prs[j] = None
                if filler:
                    filler.pop(0)()
            for t in filler:
                t()
            af = afp.tile([P, T], F16, tag=f"af{h}", name=f"af_{b}_{h}")
            for ch in range(2):
                rec = ap_.tile([P, 512], F32, tag="rec", bufs=4,
                               name=f"rec_{b}_{h}_{ch}")
                nc.vector.reciprocal_approx_fast(out=rec[:], in_=den[ch][:])
                nc.vector.scalar_tensor_tensor(
                    af[:, _cs(ch)], at[ch][:], 1.0, rec[:],
                    ALU.mult, ALU.mult,
                )
            attnf.append(af)

        inner.close()  # free xn / wqk / attention pools before o-proj

        # o-projection partial + residual; writes on the SWDGE ring so the
        # sync ring stays free; 2 AllReduce chunks per batch
        with ExitStack() as ph:
            osp = ph.enter_context(tc.tile_pool(name=f"op_{b}", bufs=1))
            pp = ph.enter_context(
                tc.tile_pool(name=f"op_ps_{b}", bufs=1, space="PSUM")
            )
            wo_sb = []
            for h in range(NH):
                wt = osp.tile([P, NDT, P], F16, tag=f"wo{h}", name=f"wo_{b}_{h}")
                nc.sync.dma_start(out=wt[:], in_=wo_t[h])
                wo_sb.append(wt)
            for dt in range(NDT):
                ps = pp.tile([P, T], F32, tag="o_ps", bufs=2, name=f"pso_{b}_{dt}")
                for h in range(NH):
                    for ch in range(T // 512):
                        nc.tensor.matmul(
                            ps[:, _cs(ch)],
                            wo_sb[h][:, dt, :],
                            attnf[h][:, _cs(ch)],
                            start=(h == 0),
                            stop=(h == NH - 1),
                        )
                # residual x is added after the AllReduce (in the MLP block);
                # the AR payload is just the o-projection partial
                osb = osp.tile([P, T], F16, tag="osb", bufs=3, name=f"osb_{b}_{dt}")
                nc.scalar.copy(osb[:], ps[:])
                nc.gpsimd.dma_start(
                    out=cc_in[b][dt * P : (dt + 1) * P, :], in_=osb[:]
                )
                if dt in (NDT // 2 - 1, NDT - 1):
                    k = dt // (NDT // 2)
                    rows = slice(k * (D // 2), (dt + 1) * P)
                    nc.gpsimd.collective_compute(
                        "AllReduce",
                        ALU.add,
                        replica_groups=[list(range(N_CORES))],
                        ins=[cc_in[b][rows, :]],
                        outs=[cc_half[b][k][:, :]],
                    )


def _mlp_batch(nc, tc, b, xT_h, cc_half, ones_h, eps_sb, bcast2, wf_t, wu_t,
               wout_t, y):
    bs = slice(b * T, (b + 1) * T)
    with ExitStack() as bph:
        bp = bph.enter_context(tc.tile_pool(name=f"mlpb_{b}", bufs=1))
        stp = bph.enter_context(tc.tile_pool(name=f"mstat_{b}", bufs=1))
        spp = bph.enter_context(
            tc.tile_pool(name=f"mstat_ps_{b}", bufs=1, space="PSUM")
        )
        sp = bph.enter_context(tc.tile_pool(name=f"mlp_{b}", bufs=1))
        pp = bph.enter_context(
            tc.tile_pool(name=f"mlp_ps_{b}", bufs=1, space="PSUM")
        )
        # x_mid = AllReduce(o-partial) + x, built once per tile and reused by
        # stats, ff/up matmuls and the wout residual.  The m=0 ff/up matmuls
        # are interleaved into this loop so the PE has dense work while the
        # x_mid tiles stream in.
        wt0 = {}
        ps0 = {}
        for which, wsrc in (("f", wf_t), ("u", wu_t)):
            wt = sp.tile([P, NKP, P], F16, tag="wffu", bufs=3,
                         name=f"w{which}_{b}_0")
            nc.sync.dma_start(out=wt[:], in_=wsrc[0])
            wt0[which] = wt
            ps0[which] = pp.tile([P, T], F32, tag="ps_fu", bufs=2,
                                 name=f"ps{which}_{b}_0")
        xmh = []
        ms_ps = spp.tile([P, T], F32, name=f"ms_{b}")
        for kp in range(NKP):
            xc = stp.tile([P, T], F16, tag="xc", bufs=3, name=f"xc_{b}_{kp}")
            half, loc = divmod(kp, NKP // 2)
            nc.sync.dma_start(
                out=xc[:], in_=cc_half[b][half][loc * P : (loc + 1) * P, :]
            )
            xr = stp.tile([P, T], F16, tag="xr", bufs=3, name=f"xr_{b}_{kp}")
            nc.sync.dma_start(
                out=xr[:], in_=xT_h[kp * P : (kp + 1) * P, bs]
            )
            xk = bp.tile([P, T], F16, tag=f"xm{kp}", name=f"xmh_{b}_{kp}")
            nc.vector.tensor_add(xk[:], xc[:], xr[:])
            xmh.append(xk)
            sq = stp.tile([P, T], F16, tag="sq", bufs=3, name=f"sq_{b}_{kp}")
            nc.scalar.activation(sq[:], xk[:], AF.Square)
            for ch in range(2):
                nc.tensor.matmul(
                    ms_ps[:, _cs(ch)], ones_h[:], sq[:, _cs(ch)],
                    start=(kp == 0), stop=(kp == NKP - 1),
                )
            for which in ("f", "u"):
                for ch in range(2):
                    nc.tensor.matmul(
                        ps0[which][:, _cs(ch)],
                        wt0[which][:, kp, :],
                        xk[:, _cs(ch)],
                        start=(kp == 0),
                        stop=(kp == NKP - 1),
                    )
        lnt = stp.tile([P, T], F32, name=f"lnt_{b}")
        nc.scalar.activation(
            lnt[:], ms_ps[:], AF.Ln, bias=eps_sb[:], scale=1.0 / D
        )
        nc.scalar.activation(bcast2[b][:], lnt[:], AF.Exp, scale=-0.5)

        hsb = []
        with ExitStack() as ph:
            ffs = []
            for m in range(NFT):
                for which, wsrc in (("f", wf_t), ("u", wu_t)):
                    if m == 0:
                        ps = ps0[which]
                    else:
                        wt = sp.tile([P, NKP, P], F16, tag="wffu", bufs=3,
                                     name=f"w{which}_{b}_{m}")
                        nc.sync.dma_start(out=wt[:], in_=wsrc[m])
                        ps = pp.tile([P, T], F32, tag="ps_fu", bufs=2,
                                     name=f"ps{which}_{b}_{m}")
                        for kp in range(NKP):
                            for ch in range(2):
                                nc.tensor.matmul(
                                    ps[:, _cs(ch)],
                                    wt[:, kp, :],
                                    xmh[kp][:, _cs(ch)],
                                    start=(kp == 0),
                                    stop=(kp == NKP - 1),
                                )
                    # fold the norm2 scale into the eviction
                    nt = sp.tile([P, T], F16, tag=f"nrm_{which}", bufs=3,
                                 name=f"nt{which}_{b}_{m}")
                    nc.vector.scalar_tensor_tensor(
                        nt[:], ps[:], 1.0, bcast2[b][:], ALU.mult, ALU.mult,
                    )
                    if which == "f":
                        ft = sp.tile([P, T], F16, tag="ffs", bufs=3,
                                     name=f"ff_{b}_{m}")
                        nc.scalar.activation(ft[:], nt[:], AF.Silu)
                        ffs.append(ft)
                    else:
                        ht = bp.tile([P, T], F16, tag=f"h{m}", name=f"h_{b}_{m}")
                        nc.vector.tensor_mul(ht[:], nt[:], ffs[m][:])
                        hsb.append(ht)

        # w_out projection + residual, partial fp16 output
        with ExitStack() as ph:
            wsp = ph.enter_context(tc.tile_pool(name=f"wo2_{b}", bufs=1))
            for dt in range(NDT):
                wt = wsp.tile([P, NFT, P], F16, tag="wot", bufs=3,
                              name=f"wot_{b}_{dt}")
                nc.sync.dma_start(out=wt[:], in_=wout_t[dt])
                ps = pp.tile([P, T], F32, tag="ps_fu", bufs=2,
                             name=f"pso2_{b}_{dt}")
                for m in range(NFT):
                    for ch in range(2):
                        nc.tensor.matmul(
                            ps[:, _cs(ch)],
                            wt[:, m, :],
                            hsb[m][:, _cs(ch)],
                            start=(m == 0),
                            stop=(m == NFT - 1),
                        )
                ysb = sp.tile([P, T], F16, tag="ysb", bufs=3,
                              name=f"ysb_{b}_{dt}")
                nc.vector.scalar_tensor_tensor(
                    ysb[:], xmh[dt][:], 1.0 / N_CORES, ps[:],
                    ALU.mult, ALU.add,
                )
                nc.sync.dma_start(out=y[dt * P : (dt + 1) * P, bs], in_=ysb[:])


_NC_CACHE = {}


def _get_nc():
    if "nc" not in _NC_CACHE:
        _NC_CACHE["nc"] = _build()
    return _NC_CACHE["nc"]


def _host_prep(x, sin, cos, attn_norm_w, ff_norm_w, wq, wk, wv, wo, w_ff, w_up, w_out):
    f16 = np.float16
    x2 = np.asarray(x, np.float32).reshape(M, D)
    xT = np.ascontiguousarray(x2.T)
    rs1 = 1.0 / np.sqrt((xT * xT).mean(0) + EPS)        # [M] norm1 on host
    xnT = xT * rs1[None, :]

    sinT = np.asarray(sin, np.float32).reshape(M, HALF).T
    cosT = np.asarray(cos, np.float32).reshape(M, HALF).T
    cc = np.concatenate([cosT, cosT], axis=0)
    ss = np.concatenate([sinT, sinT], axis=0)
    css = np.stack([cc, ss]).astype(f16)

    anw = np.asarray(attn_norm_w, np.float32)[:, None]
    fnw = np.asarray(ff_norm_w, np.float32)[:, None]
    wqn = (anw * np.asarray(wq, np.float32)) * (H ** -0.5)
    wkn = anw * np.asarray(wk, np.float32)
    wvn = anw * np.asarray(wv, np.float32)
    wfn = fnw * np.asarray(w_ff, np.float32)
    wun = fnw * np.asarray(w_up, np.float32)
    wo = np.asarray(wo, np.float32)
    w_out = np.asarray(w_out, np.float32)

    def mtile(w):
        # [K, F] -> [F/P, P, K/P, P] with [m, p, kp, j] = w[kp*P+p, m*P+j]
        K, F = w.shape
        return np.ascontiguousarray(
            w.reshape(K // P, P, F // P, P).transpose(2, 1, 0, 3)
        )

    in_maps = []
    for c in range(N_CORES):
        qs = slice(c * QC, (c + 1) * QC)
        fs = slice(c * FC, (c + 1) * FC)
        in_maps.append(
            {
                "xT_h": xT.astype(f16),
                "xnT_h": xnT.astype(f16),
                "css": css,
                "wq_t": mtile(wqn[:, qs]).astype(f16),
                "wk_t": mtile(wkn[:, qs]).astype(f16),
                "wv_n": wvn[:, qs].astype(f16),
                # [h, p, dt, j] = wo[c*QC + h*P + p, dt*P + j]
                "wo_t": np.ascontiguousarray(
                    wo[qs, :].reshape(NH, P, NDT, P)
                ).astype(f16),
                "wf_t": mtile(wfn[:, fs]).astype(f16),
                "wu_t": mtile(wun[:, fs]).astype(f16),
                "wout_t": mtile(w_out[fs, :]).astype(f16),
            }
        )
    return in_maps


def kernel(**inputs) -> np.ndarray:
    nc = _get_nc()
    in_maps = _host_prep(**inputs)
    res = run_bass_kernel_spmd(
        nc, in_maps, core_ids=list(range(N_CORES)), trace=False
    )
    acc = res.results[0]["y"].astype(np.float64)
    for c in range(1, N_CORES):
        acc += res.results[c]["y"]
    return np.ascontiguousarray(acc.T).astype(np.float32).reshape(B, T, D)


# revision 18
# speedup vs baseline: 1.0362x; 1.0130x over previous
"""TP-8 Trainium2 Bass kernel for a LLaDA/Llama transformer block.

Shapes (hardcoded): x [2, 1024, 4096], 32 heads x 128 head_dim,
FF=12288, non-causal attention, RMSNorm + RoPE + SwiGLU.

Sharding (per sharding_hint): tensor-parallel over the 8 cores —
q/k/v/ff sharded on the output-feature axis (4 heads / 1536 ff dims per
core); the o-projection is sharded on its OUTPUT rows (each core
computes the full o for D/8 rows from the AllGathered attention
outputs).  The final w_out partials are summed on the host.

v5 collective scheme (replaces the 2x8.4MB fp16 AllReduce of v2-v4,
whose DMA traffic stalled concurrent weight streams for ~250us):
 - AllGather the per-core attention outputs (1MB/rank) -> every core
   holds all 32 heads' outputs.
 - Each core computes o = attn_all @ wo[:, shard] for its 512 output
   rows (same matmul count as the old partial-sum o-proj), adds its
   x shard, and computes partial norm2 stats.
 - AllGather the x_mid shards (1MB/rank) + a tiny [P,T] fp32
   AllReduce of the stats partials.
Total wire per batch: ~2.5MB/rank vs ~17MB/rank before, and the o-sum
now accumulates in fp32 PSUM instead of an fp16 CCE tree.

Other structure (from v2-v4): norm1 on host (pre-normalized xnT_h),
fp16 everywhere on the PE, RoPE fused into the q/k PSUM eviction,
head-interleaved attention (q/k projections of head h+1 emitted as
filler between attention chunks of head h), 2-head weight prefetch,
residual/collective writes on the SWDGE ring, fast DVE reciprocal,
m=0 ff/up matmuls interleaved into the x_mid load loop, fp16 output.

Emission order hides the collective latencies: batch-1's xn/v/qk0
covers AG1(batch0); batch-1's heads cover AG2/statsAR(batch0); MLP
hides batch-1's AG2/statsAR.
"""

from contextlib import ExitStack

import numpy as np

import concourse.mybir as mybir
import concourse.tile as tile
from concourse import bacc
from concourse.bass_utils import run_bass_kernel_spmd

F32 = mybir.dt.float32
F16 = mybir.dt.float16
AF = mybir.ActivationFunctionType
ALU = mybir.AluOpType

N_CORES = 8
P = 128
B, T, D, FF = 2, 1024, 4096, 12288
M = B * T            # 2048 tokens
H = 128              # head dim
HALF = 64
QC = D // N_CORES    # 512 per-core q/k/v features (4 heads)
NH = QC // H         # 4 heads per core
FC = FF // N_CORES   # 1536 per-core ff features
NKP = D // P         # 32 K-tiles over D
NFT = FC // P        # 12 M-tiles over per-core FF
NDT = D // P         # 32 D-tiles
NST = T // P         # 8 sequence tiles per batch
EPS = 1e-05
LA = 2               # attention pipeline lookahead (512-col chunks)
RG = [list(range(N_CORES))]


def _cs(ch):
    return slice(ch * 512, (ch + 1) * 512)


def _build():
    nc = bacc.Bacc("TRN2", target_bir_lowering=False, num_devices=N_CORES)

    xnT_h = nc.declare_dram_parameter("xnT_h", [D, M], F16, isOutput=False)
    xsh_h = nc.declare_dram_parameter("xsh_h", [QC, M], F16, isOutput=False)
    css = nc.declare_dram_parameter("css", [2, P, M], F16, isOutput=False)
    wq_t = nc.declare_dram_parameter("wq_t", [NH, P, NKP, P], F16, isOutput=False)
    wk_t = nc.declare_dram_parameter("wk_t", [NH, P, NKP, P], F16, isOutput=False)
    wv_n = nc.declare_dram_parameter("wv_n", [D, QC], F16, isOutput=False)
    wosh_t = nc.declare_dram_parameter("wosh_t", [NKP, P, NH, P], F16, isOutput=False)
    wf_t = nc.declare_dram_parameter("wf_t", [NFT, P, NKP, P], F16, isOutput=False)
    wu_t = nc.declare_dram_parameter("wu_t", [NFT, P, NKP, P], F16, isOutput=False)
    wout_t = nc.declare_dram_parameter("wout_t", [NDT, P, NFT, P], F16, isOutput=False)
    y = nc.declare_dram_parameter("y", [D, M], F16, isOutput=True)

    with tile.TileContext(nc) as tc:
        _emit(nc, tc, xnT_h, xsh_h, css, wq_t, wk_t, wv_n, wosh_t, wf_t, wu_t,
              wout_t, y)
    nc.compile()
    return nc


def _emit(nc, tc, xnT_h, xsh_h, css, wq_t, wk_t, wv_n, wosh_t, wf_t, wu_t,
          wout_t, y):
    with ExitStack() as top:
        dram_pool = top.enter_context(tc.tile_pool(name="dram", bufs=1, space="DRAM"))
        const = top.enter_context(tc.tile_pool(name="const", bufs=1))

        att_in = [dram_pool.tile([QC, T], F16, name=f"att_in_{b}") for b in range(B)]
        att_all = [
            dram_pool.tile([D, T], F16, addr_space="Shared", name=f"att_all_{b}")
            for b in range(B)
        ]
        xm_in = [dram_pool.tile([QC, T], F16, name=f"xm_in_{b}") for b in range(B)]
        xm_all = [
            dram_pool.tile([D, T], F16, addr_space="Shared", name=f"xm_all_{b}")
            for b in range(B)
        ]
        st_in = [dram_pool.tile([P, T], F32, name=f"st_in_{b}") for b in range(B)]
        st_out = [
            dram_pool.tile([P, T], F32, addr_space="Shared", name=f"st_out_{b}")
            for b in range(B)
        ]

        ones_h = const.tile([P, P], F16)
        nc.vector.memset(ones_h[:], 1.0)
        cc_sb = const.tile([P, M], F16)
        ss_sb = const.tile([P, M], F16)
        nc.sync.dma_start(out=cc_sb[:], in_=css[0])
        nc.sync.dma_start(out=ss_sb[:], in_=css[1])
        eps_sb = const.tile([P, 1], F32)
        nc.vector.memset(eps_sb[:], EPS)
        bcast2 = [const.tile([P, T], F16, name=f"bcast2_{b}") for b in range(B)]

        cst = (ones_h, cc_sb, ss_sb)

        g0 = _attn_gen(nc, tc, 0, xnT_h, wq_t, wk_t, wv_n, att_in, att_all, cst)
        next(g0)                    # b0: xn + v + head-0 projections
        next(g0, None)              # b0: heads + AG1(0)
        g1 = _attn_gen(nc, tc, 1, xnT_h, wq_t, wk_t, wv_n, att_in, att_all, cst)
        next(g1)                    # b1 phase1 — covers AG1(0) latency
        _oproj_block(nc, tc, 0, xsh_h, wosh_t, att_all, xm_in, xm_all,
                     st_in, st_out, ones_h)
        next(g1, None)              # b1 heads + AG1(1) — covers AG2/sAR(0)
        _oproj_block(nc, tc, 1, xsh_h, wosh_t, att_all, xm_in, xm_all,
                     st_in, st_out, ones_h)
        for b in range(B):
            _mlp_batch(nc, tc, b, xm_all, st_out, eps_sb, bcast2, wf_t, wu_t,
                       wout_t, y)


def _attn_gen(nc, tc, b, xnT_h, wq_t, wk_t, wv_n, att_in, att_all, cst):
    ones_h, cc_sb, ss_sb = cst
    bs = slice(b * T, (b + 1) * T)
    with ExitStack() as bph:
        xp = bph.enter_context(tc.tile_pool(name=f"xn_{b}", bufs=1))
        sp = bph.enter_context(tc.tile_pool(name=f"qkv_{b}", bufs=1))

        # v projection (token-major); xn and wv DMAs interleave so the
        # first matmul starts immediately
        xn = []
        v_sb = []
        with ExitStack() as vph:
            vpp = vph.enter_context(
                tc.tile_pool(name=f"v_ps_{b}", bufs=1, space="PSUM")
            )
            ps_v = [
                vpp.tile([P, QC], F32, tag=f"vps{st}", name=f"psv_{b}_{st}")
                for st in range(NST)
            ]
            for kp in range(NKP):
                xnk = xp.tile([P, T], F16, tag=f"xn{kp}", name=f"xn_{b}_{kp}")
                nc.sync.dma_start(
                    out=xnk[:], in_=xnT_h[kp * P : (kp + 1) * P, bs]
                )
                xn.append(xnk)
                wvk = sp.tile([P, QC], F16, tag="wv", bufs=3, name=f"wv_{b}_{kp}")
                nc.sync.dma_start(
                    out=wvk[:], in_=wv_n[kp * P : (kp + 1) * P, :]
                )
                for st in range(NST):
                    nc.tensor.matmul(
                        ps_v[st][:],
                        xn[kp][:, st * P : (st + 1) * P],
                        wvk[:],
                        start=(kp == 0),
                        stop=(kp == NKP - 1),
                    )
            for st in range(NST):
                vt = xp.tile([P, QC], F16, tag=f"v{st}", name=f"v_{b}_{st}")
                nc.scalar.copy(vt[:], ps_v[st][:])
                v_sb.append(vt)

        # q/k projection thunks: 8 filler-sized pieces per (which, head),
        # rope eviction inside the last piece; weights prefetch 2 heads ahead
        qpp = bph.enter_context(tc.tile_pool(name=f"qk_ps_{b}", bufs=1, space="PSUM"))
        qf, kf = [None] * NH, [None] * NH
        wts = {}

        def ensure_w(h):
            if h >= NH or ("q", h) in wts:
                return
            for which, wsrc in (("q", wq_t), ("k", wk_t)):
                wt = sp.tile([P, NKP, P], F16, tag="wqk", bufs=4,
                             name=f"w{which}_{b}_{h}")
                nc.sync.dma_start(out=wt[:], in_=wsrc[h])
                wts[(which, h)] = wt

        def make_proj_thunks(which, h, dst, idx):
            wt = wts[(which, h)]
            state = {}

            def piece(i):
                def run():
                    if i == 0:
                        state["ps"] = qpp.tile(
                            [P, T], F32, tag="qk_ps", bufs=1,
                            name=f"ps{which}_{b}_{h}",
                        )
                    ps = state["ps"]
                    for kp in range(i * 4, i * 4 + 4):
                        for ch in range(2):
                            nc.tensor.matmul(
                                ps[:, _cs(ch)],
                                wt[:, kp, :],
                                xn[kp][:, _cs(ch)],
                                start=(kp == 0),
                                stop=(kp == NKP - 1),
                            )
                    if i == 7:
                        main = sp.tile([P, T], F16, tag="rmain", bufs=2,
                                       name=f"rm_{which}_{b}_{h}")
                        nc.vector.scalar_tensor_tensor(
                            main[:], ps[:], 1.0, cc_sb[:, bs],
                            ALU.mult, ALU.mult,
                        )
                        rot = sp.tile([P, T], F16, tag="rrot", bufs=2,
                                      name=f"rr_{which}_{b}_{h}")
                        nc.vector.scalar_tensor_tensor(
                            rot[:HALF], ps[HALF:], -1.0,
                            ss_sb[:HALF, bs], ALU.mult, ALU.mult,
                        )
                        nc.vector.scalar_tensor_tensor(
                            rot[HALF:], ps[:HALF], 1.0,
                            ss_sb[HALF:, bs], ALU.mult, ALU.mult,
                        )
                        out = xp.tile([P, T], F16, tag=f"{which}f{h}",
                                      name=f"{which}f_{b}_{h}")
                        nc.vector.tensor_add(out[:], main[:], rot[:])
                        dst[idx] = out
                return run

            return [piece(i) for i in range(8)]

        def head_thunks(h):
            return (make_proj_thunks("q", h, qf, h)
                    + make_proj_thunks("k", h, kf, h))

        ensure_w(0)
        ensure_w(1)
        for t in head_thunks(0):
            t()

        yield  # phase boundary: caller interleaves other batch's work here

        # attention per head, chunk-pipelined, filler = head h+1 projections
        ap_ = bph.enter_context(tc.tile_pool(name=f"att_{b}", bufs=1))
        app = bph.enter_context(tc.tile_pool(name=f"att_ps_{b}", bufs=1, space="PSUM"))
        for h in range(NH):
            ensure_w(h + 2)
            filler = head_thunks(h + 1) if h + 1 < NH else []
            den = [
                app.tile([P, 512], F32, tag=f"den{ch}", name=f"den_{b}_{h}_{ch}")
                for ch in range(2)
            ]
            at = [
                app.tile([P, 512], F32, tag=f"at{ch}", name=f"at_{b}_{h}_{ch}")
                for ch in range(2)
            ]

            def emit_lg(j, h=h):
                st, ch = divmod(j, 2)
                lg = app.tile([P, 512], F32, tag="lg", bufs=2,
                              name=f"lg_{b}_{h}_{j}")
                nc.tensor.matmul(
                    lg[:],
                    kf[h][:, st * P : (st + 1) * P],
                    qf[h][:, _cs(ch)],
                    start=True,
                    stop=True,
                )
                pr = ap_.tile([P, 512], F16, tag="pr", bufs=6,
                              name=f"pr_{b}_{h}_{j}")
                nc.scalar.activation(pr[:], lg[:], AF.Exp)
                return pr

            prs = [None] * 16
            for j in range(LA):
                prs[j] = emit_lg(j)
            for j in range(16):
                if j + LA < 16:
                    prs[j + LA] = emit_lg(j + LA)
                st, ch = divmod(j, 2)
                pr = prs[j]
                nc.tensor.matmul(
                    den[ch][:], ones_h[:], pr[:],
                    start=(st == 0), stop=(st == NST - 1),
                )
                nc.tensor.matmul(
                    at[ch][:],
                    v_sb[st][:, h * H : (h + 1) * H],
                    pr[:],
                    start=(st == 0), stop=(st == NST - 1),
                )
                prs[j] = None
                if filler:
                    filler.pop(0)()
            for t in filler:
                t()
            af = ap_.tile([P, T], F16, tag="af", bufs=2, name=f"af_{b}_{h}")
            for ch in range(2):
                rec = ap_.tile([P, 512], F32, tag="rec", bufs=4,
                               name=f"rec_{b}_{h}_{ch}")
                nc.vector.reciprocal_approx_fast(out=rec[:], in_=den[ch][:])
                nc.vector.scalar_tensor_tensor(
                    af[:, _cs(ch)], at[ch][:], 1.0, rec[:],
                    ALU.mult, ALU.mult,
                )
            nc.gpsimd.dma_start(
                out=att_in[b][h * P : (h + 1) * P, :], in_=af[:]
            )
        nc.gpsimd.collective_compute(
            "AllGather", ALU.bypass, replica_groups=RG,
            ins=[att_in[b][:, :]], outs=[att_all[b][:, :]],
        )


def _oproj_block(nc, tc, b, xsh_h, wosh_t, att_all, xm_in, xm_all,
                 st_in, st_out, ones_h):
    """o = attn_all @ wo_shard for this core's 512 output rows, + x shard,
    + partial norm2 stats.  kp-outer with 2 row-tiles per pass (2 passes)
    keeps it at 6 PSUM banks and ~34KB/partition of SBUF."""
    bs = slice(b * T, (b + 1) * T)
    with ExitStack() as ph:
        osp = ph.enter_context(tc.tile_pool(name=f"op_{b}", bufs=1))
        pp = ph.enter_context(tc.tile_pool(name=f"op_ps_{b}", bufs=1, space="PSUM"))
        ms_ps = pp.tile([P, T], F32, name=f"ms_{b}")
        for half in range(2):
            ms = (2 * half, 2 * half + 1)
            ps = {
                m: pp.tile([P, T], F32, tag=f"o_ps{m - 2 * half}",
                           name=f"pso_{b}_{m}")
                for m in ms
            }
            for kp in range(NKP):
                atk = osp.tile([P, T], F16, tag="atk", bufs=6,
                               name=f"atk_{b}_{half}_{kp}")
                nc.sync.dma_start(
                    out=atk[:], in_=att_all[b][kp * P : (kp + 1) * P, :]
                )
                wok = osp.tile([P, NH, P], F16, tag="wok", bufs=6,
                               name=f"wok_{b}_{half}_{kp}")
                nc.sync.dma_start(out=wok[:], in_=wosh_t[kp])
                for m in ms:
                    for ch in range(2):
                        nc.tensor.matmul(
                            ps[m][:, _cs(ch)],
                            wok[:, m, :],
                            atk[:, _cs(ch)],
                            start=(kp == 0),
                            stop=(kp == NKP - 1),
                        )
            for m in ms:
                xt = osp.tile([P, T], F16, tag="xsh", bufs=2, name=f"xs_{b}_{m}")
                nc.sync.dma_start(out=xt[:], in_=xsh_h[m * P : (m + 1) * P, bs])
                osb = osp.tile([P, T], F16, tag="osb", bufs=2, name=f"osb_{b}_{m}")
                nc.vector.tensor_add(osb[:], xt[:], ps[m][:])
                nc.gpsimd.dma_start(
                    out=xm_in[b][m * P : (m + 1) * P, :], in_=osb[:]
                )
                sq = osp.tile([P, T], F16, tag="sq", bufs=2, name=f"sq_{b}_{m}")
                nc.scalar.activation(sq[:], osb[:], AF.Square)
                for ch in range(2):
                    nc.tensor.matmul(
                        ms_ps[:, _cs(ch)], ones_h[:], sq[:, _cs(ch)],
                        start=(m == 0), stop=(m == NH - 1),
                    )
        msb = osp.tile([P, T], F32, name=f"msb_{b}")
        nc.scalar.copy(msb[:], ms_ps[:])
        nc.gpsimd.dma_start(out=st_in[b][:, :], in_=msb[:])
        nc.gpsimd.collective_compute(
            "AllGather", ALU.bypass, replica_groups=RG,
            ins=[xm_in[b][:, :]], outs=[xm_all[b][:, :]],
        )
        nc.gpsimd.collective_compute(
            "AllReduce", ALU.add, replica_groups=RG,
            ins=[st_in[b][:, :]], outs=[st_out[b][:, :]],
        )


def _mlp_batch(nc, tc, b, xm_all, st_out, eps_sb, bcast2, wf_t, wu_t,
               wout_t, y):
    bs = slice(b * T, (b + 1) * T)
    with ExitStack() as bph:
        bp = bph.enter_context(tc.tile_pool(name=f"mlpb_{b}", bufs=1))
        sp = bph.enter_context(tc.tile_pool(name=f"mlp_{b}", bufs=1))
        pp = bph.enter_context(
            tc.tile_pool(name=f"mlp_ps_{b}", bufs=1, space="PSUM")
        )
        # norm2 scale from the AllReduced stats partials
        stb = sp.tile([P, T], F32, name=f"stb_{b}")
        nc.sync.dma_start(out=stb[:], in_=st_out[b][:, :])
        lnt = sp.tile([P, T], F32, name=f"lnt_{b}")
        nc.scalar.activation(lnt[:], stb[:], AF.Ln, bias=eps_sb[:], scale=1.0 / D)
        nc.scalar.activation(bcast2[b][:], lnt[:], AF.Exp, scale=-0.5)

        # x_mid tiles stream in with the m=0 ff/up matmuls interleaved so
        # the PE has dense work from the first tile
        wt0 = {}
        ps0 = {}
        for which, wsrc in (("f", wf_t), ("u", wu_t)):
            wt = sp.tile([P, NKP, P], F16, tag="wffu", bufs=3,
                         name=f"w{which}_{b}_0")
            nc.sync.dma_start(out=wt[:], in_=wsrc[0])
            wt0[which] = wt
            ps0[which] = pp.tile([P, T], F32, tag="ps_fu", bufs=2,
                                 name=f"ps{which}_{b}_0")
        xmh = []
        for kp in range(NKP):
            xk = bp.tile([P, T], F16, tag=f"xm{kp}", name=f"xmh_{b}_{kp}")
            nc.sync.dma_start(
                out=xk[:], in_=xm_all[b][kp * P : (kp + 1) * P, :]
            )
            xmh.append(xk)
            for which in ("f", "u"):
                for ch in range(2):
                    nc.tensor.matmul(
                        ps0[which][:, _cs(ch)],
                        wt0[which][:, kp, :],
                        xk[:, _cs(ch)],
                        start=(kp == 0),
                        stop=(kp == NKP - 1),
                    )

        hsb = []
        ffs = []
        for m in range(NFT):
            for which, wsrc in (("f", wf_t), ("u", wu_t)):
                if m == 0:
                    ps = ps0[which]
                else:
                    wt = sp.tile([P, NKP, P], F16, tag="wffu", bufs=3,
                                 name=f"w{which}_{b}_{m}")
                    nc.sync.dma_start(out=wt[:], in_=wsrc[m])
                    ps = pp.tile([P, T], F32, tag="ps_fu", bufs=2,
                                 name=f"ps{which}_{b}_{m}")
                    for kp in range(NKP):
                        for ch in range(2):
                            nc.tensor.matmul(
                                ps[:, _cs(ch)],
                                wt[:, kp, :],
                                xmh[kp][:, _cs(ch)],
                                start=(kp == 0),
                                stop=(kp == NKP - 1),
                            )
                # fold the norm2 scale into the eviction
                nt = sp.tile([P, T], F16, tag=f"nrm_{which}", bufs=3,
                             name=f"nt{which}_{b}_{m}")
                nc.vector.scalar_tensor_tensor(
                    nt[:], ps[:], 1.0, bcast2[b][:], ALU.mult, ALU.mult,
                )
                if which == "f":
                    ft = sp.tile([P, T], F16, tag="ffs", bufs=3,
                                 name=f"ff_{b}_{m}")
                    nc.scalar.activation(ft[:], nt[:], AF.Silu)
                    ffs.append(ft)
                else:
                    ht = bp.tile([P, T], F16, tag=f"h{m}", name=f"h_{b}_{m}")
                    nc.vector.tensor_mul(ht[:], nt[:], ffs[m][:])
                    hsb.append(ht)

        # w_out projection + residual, partial fp16 output
        with ExitStack() as ph:
            wsp = ph.enter_context(tc.tile_pool(name=f"wo2_{b}", bufs=1))
            for dt in range(NDT):
                wt = wsp.tile([P, NFT, P], F16, tag="wot", bufs=3,
                              name=f"wot_{b}_{dt}")
                nc.sync.dma_start(out=wt[:], in_=wout_t[dt])
                ps = pp.tile([P, T], F32, tag="ps_fu", bufs=2,
                             name=f"pso2_{b}_{dt}")
                for m in range(NFT):
                    for ch in range(2):
                        nc.tensor.matmul(
                            ps[:, _cs(ch)],
                            wt[:, m, :],
                            hsb[m][:, _cs(ch)],
                            start=(m == 0),
                            stop=(m == NFT - 1),
                        )
                ysb = wsp.tile([P, T], F16, tag="ysb", bufs=3,
                               name=f"ysb_{b}_{dt}")
                nc.vector.scalar_tensor_tensor(
                    ysb[:], xmh[dt][:], 1.0 / N_CORES, ps[:],
                    ALU.mult, ALU.add,
                )
                nc.sync.dma_start(out=y[dt * P : (dt + 1) * P, bs], in_=ysb[:])


_NC_CACHE = {}


def _get_nc():
    if "nc" not in _NC_CACHE:
        _NC_CACHE["nc"] = _build()
    return _NC_CACHE["nc"]


def _host_prep(x, sin, cos, attn_norm_w, ff_norm_w, wq, wk, wv, wo, w_ff, w_up, w_out):
    f16 = np.float16
    x2 = np.asarray(x, np.float32).reshape(M, D)
    xT = np.ascontiguousarray(x2.T)
    rs1 = 1.0 / np.sqrt((xT * xT).mean(0) + EPS)        # [M] norm1 on host
    xnT = xT * rs1[None, :]

    sinT = np.asarray(sin, np.float32).reshape(M, HALF).T
    cosT = np.asarray(cos, np.float32).reshape(M, HALF).T
    cc = np.concatenate([cosT, cosT], axis=0)
    ss = np.concatenate([sinT, sinT], axis=0)
    css = np.stack([cc, ss]).astype(f16)

    anw = np.asarray(attn_norm_w, np.float32)[:, None]
    fnw = np.asarray(ff_norm_w, np.float32)[:, None]
    wqn = (anw * np.asarray(wq, np.float32)) * (H ** -0.5)
    wkn = anw * np.asarray(wk, np.float32)
    wvn = anw * np.asarray(wv, np.float32)
    wfn = fnw * np.asarray(w_ff, np.float32)
    wun = fnw * np.asarray(w_up, np.float32)
    wo = np.asarray(wo, np.float32)
    w_out = np.asarray(w_out, np.float32)
    xT_h = xT.astype(f16)

    def mtile(w):
        # [K, F] -> [F/P, P, K/P, P] with [m, p, kp, j] = w[kp*P+p, m*P+j]
        K, F = w.shape
        return np.ascontiguousarray(
            w.reshape(K // P, P, F // P, P).transpose(2, 1, 0, 3)
        )

    in_maps = []
    for c in range(N_CORES):
        qs = slice(c * QC, (c + 1) * QC)
        fs = slice(c * FC, (c + 1) * FC)
        in_maps.append(
            {
                "xnT_h": xnT.astype(f16),
                "xsh_h": xT_h[qs, :],
                "css": css,
                "wq_t": mtile(wqn[:, qs]).astype(f16),
                "wk_t": mtile(wkn[:, qs]).astype(f16),
                "wv_n": wvn[:, qs].astype(f16),
                # [kp, p, m, j] = wo[kp*P+p, c*QC + m*P + j]
                "wosh_t": np.ascontiguousarray(
                    wo[:, qs].reshape(NKP, P, NH, P)
                ).astype(f16),
                "wf_t": mtile(wfn[:, fs]).astype(f16),
                "wu_t": mtile(wun[:, fs]).astype(f16),
                "wout_t": mtile(w_out[fs, :]).astype(f16),
            }
        )
    return in_maps


def kernel(**inputs) -> np.ndarray:
    nc = _get_nc()
    in_maps = _host_prep(**inputs)
    res = run_bass_kernel_spmd(
        nc, in_maps, core_ids=list(range(N_CORES)), trace=False
    )
    acc = res.results[0]["y"].astype(np.float64)
    for c in range(1, N_CORES):
        acc += res.results[c]["y"]
    return np.ascontiguousarray(acc.T).astype(np.float32).reshape(B, T, D)


# revision 25
# speedup vs baseline: 1.0794x; 1.0417x over previous
"""TP-8 Trainium2 Bass kernel for a LLaDA/Llama transformer block.

Shapes (hardcoded): x [2, 1024, 4096], 32 heads x 128 head_dim,
FF=12288, non-causal attention, RMSNorm + RoPE + SwiGLU.

Sharding: tensor-parallel over 8 cores — q/k/v/ff sharded on the
output-feature axis (4 heads / 1536 ff dims per core); the
o-projection is sharded on its OUTPUT rows (each core computes the
full o for D/8 rows from AllGathered attention outputs).  w_out
partials are summed on the host.

Collectives per batch (all small): AllGather of the per-core attention
outputs in 2 chunks (0.5MB/rank each, fired after head 1 / head 3),
AllGather of the x_mid shards (1MB/rank), and a [P,T] fp32 AllReduce
of the norm2 stats partials — ~2.5MB/rank total vs the 17MB/rank
AllReduce scheme this replaced.

Overlap structure (v6): attention input tiles and q/k weights live in
pools SHARED across batches (tag rings), so batch-1's loads fire
deterministically as batch-0 frees each tile; batch-1's v/qk covers
AG1(b0); an MLP prefix (x_mid loads + the m=0 ff/up matmuls) is
emitted before oproj(b1) to cover AG1(b1); RoPE is fused into the q/k
PSUM eviction; attention is chunk-pipelined with the next head's
projection matmuls as filler; norm1 is precomputed on the host; all
residual/collective writes ride the SWDGE ring.
"""

from contextlib import ExitStack

import numpy as np

import concourse.mybir as mybir
import concourse.tile as tile
from concourse import bacc
from concourse.bass_utils import run_bass_kernel_spmd

F32 = mybir.dt.float32
F16 = mybir.dt.float16
AF = mybir.ActivationFunctionType
ALU = mybir.AluOpType

N_CORES = 8
P = 128
B, T, D, FF = 2, 1024, 4096, 12288
M = B * T            # 2048 tokens
H = 128              # head dim
HALF = 64
QC = D // N_CORES    # 512 per-core q/k/v features (4 heads)
NH = QC // H         # 4 heads per core
FC = FF // N_CORES   # 1536 per-core ff features
NKP = D // P         # 32 K-tiles over D
NFT = FC // P        # 12 M-tiles over per-core FF
NDT = D // P         # 32 D-tiles
NST = T // P         # 8 sequence tiles per batch
EPS = 1e-05
LA = 2               # attention pipeline lookahead (512-col chunks)
RG = [list(range(N_CORES))]


def _cs(ch):
    return slice(ch * 512, (ch + 1) * 512)


def _build():
    nc = bacc.Bacc("TRN2", target_bir_lowering=False, num_devices=N_CORES)

    dp = {}
    dp["xnT_h"] = nc.declare_dram_parameter("xnT_h", [D, M], F16, isOutput=False)
    dp["xsh_h"] = nc.declare_dram_parameter("xsh_h", [QC, M], F16, isOutput=False)
    dp["css"] = nc.declare_dram_parameter("css", [2, P, M], F16, isOutput=False)
    dp["wq_t"] = nc.declare_dram_parameter("wq_t", [NH, P, NKP, P], F16, isOutput=False)
    dp["wk_t"] = nc.declare_dram_parameter("wk_t", [NH, P, NKP, P], F16, isOutput=False)
    dp["wv_n"] = nc.declare_dram_parameter("wv_n", [D, QC], F16, isOutput=False)
    dp["wosh_t"] = nc.declare_dram_parameter("wosh_t", [NKP, P, NH, P], F16, isOutput=False)
    dp["wf_t"] = nc.declare_dram_parameter("wf_t", [NFT, P, NKP, P], F16, isOutput=False)
    dp["wu_t"] = nc.declare_dram_parameter("wu_t", [NFT, P, NKP, P], F16, isOutput=False)
    dp["wout_t"] = nc.declare_dram_parameter("wout_t", [NDT, P, NFT, P], F16, isOutput=False)
    dp["y"] = nc.declare_dram_parameter("y", [D, M], F16, isOutput=True)

    with tile.TileContext(nc) as tc:
        _emit(nc, tc, dp)
    nc.compile()
    return nc


def _emit(nc, tc, dp):
    with ExitStack() as top:
        dram = top.enter_context(tc.tile_pool(name="dram", bufs=1, space="DRAM"))
        const = top.enter_context(tc.tile_pool(name="const", bufs=1))

        dr = {}
        for b in range(B):
            dr[("att_in", b)] = dram.tile([QC, T], F16, name=f"att_in_{b}")
            dr[("att01", b)] = dram.tile([D // 2, T], F16, addr_space="Shared",
                                         name=f"att01_{b}")
            dr[("att23", b)] = dram.tile([D // 2, T], F16, addr_space="Shared",
                                         name=f"att23_{b}")
            dr[("xm_in", b)] = dram.tile([QC, T], F16, name=f"xm_in_{b}")
            dr[("xm_all", b)] = dram.tile([D, T], F16, addr_space="Shared",
                                          name=f"xm_all_{b}")
            dr[("st_in", b)] = dram.tile([P, T], F32, name=f"st_in_{b}")
            dr[("st_out", b)] = dram.tile([P, T], F32, addr_space="Shared",
                                          name=f"st_out_{b}")

        cn = {}
        cn["ones"] = const.tile([P, P], F16, name="ones_h")
        nc.vector.memset(cn["ones"][:], 1.0)
        cn["cc"] = const.tile([P, M], F16, name="cc_sb")
        cn["ss"] = const.tile([P, M], F16, name="ss_sb")
        cn["eps"] = const.tile([P, 1], F32, name="eps_sb")
        nc.vector.memset(cn["eps"][:], EPS)
        cn["bcast2"] = [const.tile([P, T], F16, name=f"bcast2_{b}") for b in range(B)]

        with ExitStack() as asec:
            # pools shared across both batches: tag rings double-buffer
            # cross-batch so batch-1 loads fire as batch-0 frees tiles
            px = asec.enter_context(tc.tile_pool(name="px", bufs=1))
            pw = asec.enter_context(tc.tile_pool(name="pw", bufs=1))
            pa = asec.enter_context(tc.tile_pool(name="pa", bufs=1))
            qpp = asec.enter_context(tc.tile_pool(name="qk_ps", bufs=1, space="PSUM"))
            pools = (px, pw, pa, qpp)

            c0 = _p1(nc, tc, 0, pools, dp, cn)
            _heads(nc, tc, 0, pools, dp, cn, dr, c0)
            c1 = _p1(nc, tc, 1, pools, dp, cn)
            _oproj(nc, tc, 0, dp, cn, dr)
            _heads(nc, tc, 1, pools, dp, cn, dr, c1)

        m0 = _mlp_begin(nc, tc, 0, dp, cn, dr)
        _oproj(nc, tc, 1, dp, cn, dr)
        _mlp_finish(nc, tc, 0, dp, cn, m0)
        m1 = _mlp_begin(nc, tc, 1, dp, cn, dr)
        _mlp_finish(nc, tc, 1, dp, cn, m1)


def _p1(nc, tc, b, pools, dp, cn):
    """xn + wv loads (interleaved), v-projection in 2 passes of 4 seq
    tiles (4 PSUM banks), weight prefetch, head-0 q/k projections."""
    px, pw, pa, qpp = pools
    bs = slice(b * T, (b + 1) * T)
    ctx = {"b": b, "qf": [None] * NH, "kf": [None] * NH, "wts": {},
           "xn": [], "v": []}
    xn, v_sb = ctx["xn"], ctx["v"]

    for kp in range(NKP):
        xnk = px.tile([P, T], F16, tag=f"xn{kp}", name=f"xn_{b}_{kp}")
        nc.sync.dma_start(out=xnk[:], in_=dp["xnT_h"][kp * P : (kp + 1) * P, bs])
        xn.append(xnk)
    if b == 0:
        nc.sync.dma_start(out=cn["cc"][:], in_=dp["css"][0])
        nc.sync.dma_start(out=cn["ss"][:], in_=dp["css"][1])

    # wv is re-streamed per pass (the 8-deep ring can't hold all 32 tiles
    # across two passes without deadlocking)
    for g in range(2):
        sts = range(g * 4, g * 4 + 4)
        with ExitStack() as vph:
            vpp = vph.enter_context(
                tc.tile_pool(name=f"v_ps_{b}_{g}", bufs=1, space="PSUM")
            )
            ps_v = {
                st: vpp.tile([P, QC], F32, tag=f"vps{st % 4}",
                             name=f"psv_{b}_{st}")
                for st in sts
            }
            for kp in range(NKP):
                wvk = pw.tile([P, QC], F16, tag="wv", bufs=8,
                              name=f"wv_{b}_{g}_{kp}")
                nc.sync.dma_start(
                    out=wvk[:], in_=dp["wv_n"][kp * P : (kp + 1) * P, :]
                )
                for st in sts:
                    nc.tensor.matmul(
                        ps_v[st][:],
                        xn[kp][:, st * P : (st + 1) * P],
                        wvk[:],
                        start=(kp == 0),
                        stop=(kp == NKP - 1),
                    )
            for st in sts:
                vt = pa.tile([P, QC], F16, tag=f"v{st}", name=f"v_{b}_{st}")
                nc.scalar.copy(vt[:], ps_v[st][:])
                v_sb.append(vt)

    def ensure_w(h):
        if h >= NH or ("q", h) in ctx["wts"]:
            return
        for which, wsrc in (("q", dp["wq_t"]), ("k", dp["wk_t"])):
            wt = pw.tile([P, NKP, P], F16, tag="wqk", bufs=4,
                         name=f"w{which}_{b}_{h}")
            nc.sync.dma_start(out=wt[:], in_=wsrc[h])
            ctx["wts"][(which, h)] = wt

    def make_proj_thunks(which, h, dst):
        wt = ctx["wts"][(which, h)]
        state = {}

        def piece(i):
            def run():
                if i == 0:
                    state["ps"] = qpp.tile(
                        [P, T], F32, tag="qk_ps", bufs=1,
                        name=f"ps{which}_{b}_{h}",
                    )
                ps = state["ps"]
                for kp in range(i * 4, i * 4 + 4):
                    for ch in range(2):
                        nc.tensor.matmul(
                            ps[:, _cs(ch)],
                            wt[:, kp, :],
                            xn[kp][:, _cs(ch)],
                            start=(kp == 0),
                            stop=(kp == NKP - 1),
                        )
                if i == 7:
                    main = pw.tile([P, T], F16, tag="rmain", bufs=2,
                                   name=f"rm_{which}_{b}_{h}")
                    nc.vector.scalar_tensor_tensor(
                        main[:], ps[:], 1.0, cn["cc"][:, bs],
                        ALU.mult, ALU.mult,
                    )
                    rot = pw.tile([P, T], F16, tag="rrot", bufs=2,
                                  name=f"rr_{which}_{b}_{h}")
                    nc.vector.scalar_tensor_tensor(
                        rot[:HALF], ps[HALF:], -1.0,
                        cn["ss"][:HALF, bs], ALU.mult, ALU.mult,
                    )
                    nc.vector.scalar_tensor_tensor(
                        rot[HALF:], ps[:HALF], 1.0,
                        cn["ss"][HALF:, bs], ALU.mult, ALU.mult,
                    )
                    out = pa.tile([P, T], F16, tag=f"{which}f{h}",
                                  name=f"{which}f_{b}_{h}")
                    nc.vector.tensor_add(out[:], main[:], rot[:])
                    dst[h] = out
            return run

        return [piece(i) for i in range(8)]

    def head_thunks(h):
        return (make_proj_thunks("q", h, ctx["qf"])
                + make_proj_thunks("k", h, ctx["kf"]))

    ctx["ensure_w"] = ensure_w
    ctx["head_thunks"] = head_thunks
    ensure_w(0)
    ensure_w(1)
    for t in head_thunks(0):
        t()
    return ctx


def _heads(nc, tc, b, pools, dp, cn, dr, ctx):
    """Attention per head, chunk-pipelined (lg -> exp -> den/pv with
    lookahead), next head's projections as filler; AllGather of the
    attention outputs in 2 chunks (after head 1 and head 3)."""
    px, pw, pa, qpp = pools
    qf, kf, v_sb = ctx["qf"], ctx["kf"], ctx["v"]
    ones_h = cn["ones"]
    with ExitStack() as ah:
        app = ah.enter_context(
            tc.tile_pool(name=f"att_ps_{b}", bufs=1, space="PSUM")
        )
        for h in range(NH):
            ctx["ensure_w"](h + 2)
            filler = ctx["head_thunks"](h + 1) if h + 1 < NH else []
            den = [
                app.tile([P, 512], F32, tag=f"den{ch}", name=f"den_{b}_{h}_{ch}")
                for ch in range(2)
            ]
            at = [
                app.tile([P, 512], F32, tag=f"at{ch}", name=f"at_{b}_{h}_{ch}")
                for ch in range(2)
            ]

            def emit_lg(j, h=h):
                st, ch = divmod(j, 2)
                lg = app.tile([P, 512], F32, tag="lg", bufs=2,
                              name=f"lg_{b}_{h}_{j}")
                nc.tensor.matmul(
                    lg[:],
                    kf[h][:, st * P : (st + 1) * P],
                    qf[h][:, _cs(ch)],
                    start=True,
                    stop=True,
                )
                pr = pa.tile([P, 512], F16, tag="pr", bufs=6,
                             name=f"pr_{b}_{h}_{j}")
                nc.scalar.activation(pr[:], lg[:], AF.Exp)
                return pr

            prs = [None] * 16
            for j in range(LA):
                prs[j] = emit_lg(j)
            for j in range(16):
                if j + LA < 16:
                    prs[j + LA] = emit_lg(j + LA)
                st, ch = divmod(j, 2)
                pr = prs[j]
                nc.tensor.matmul(
                    den[ch][:], ones_h[:], pr[:],
                    start=(st == 0), stop=(st == NST - 1),
                )
                nc.tensor.matmul(
                    at[ch][:],
                    v_sb[st][:, h * H : (h + 1) * H],
                    pr[:],
                    start=(st == 0), stop=(st == NST - 1),
                )
                prs[j] = None
                if filler:
                    filler.pop(0)()
            for t in filler:
                t()
            af = pa.tile([P, T], F16, tag="af", bufs=2, name=f"af_{b}_{h}")
            for ch in range(2):
                rec = pa.tile([P, 512], F32, tag="rec", bufs=4,
                              name=f"rec_{b}_{h}_{ch}")
                nc.vector.reciprocal_approx_fast(out=rec[:], in_=den[ch][:])
                nc.vector.scalar_tensor_tensor(
                    af[:, _cs(ch)], at[ch][:], 1.0, rec[:],
                    ALU.mult, ALU.mult,
                )
            nc.gpsimd.dma_start(
                out=dr[("att_in", b)][h * P : (h + 1) * P, :], in_=af[:]
            )
            if h in (1, 3):
                k = h // 2
                nc.gpsimd.collective_compute(
                    "AllGather", ALU.bypass, replica_groups=RG,
                    ins=[dr[("att_in", b)][k * 2 * P : (k + 1) * 2 * P, :]],
                    outs=[dr[(f"att{2 * k}{2 * k + 1}", b)][:, :]],
                )


def _oproj(nc, tc, b, dp, cn, dr):
    """o = attn_all @ wo_shard for this core's 512 output rows (+x shard,
    +partial norm2 stats).  kp-outer, 2 row-tiles per pass, 2 passes:
    6 PSUM banks, ~28KB/partition SBUF, attention stream read twice."""
    bs = slice(b * T, (b + 1) * T)

    def att_rows(kp):
        c, h = divmod(kp, NH)
        src = dr[(f"att{(h // 2) * 2}{(h // 2) * 2 + 1}", b)]
        base = c * 2 * P + (h % 2) * P
        return src[base : base + P, :]

    with ExitStack() as ph:
        osp = ph.enter_context(tc.tile_pool(name=f"op_{b}", bufs=1))
        pp = ph.enter_context(tc.tile_pool(name=f"op_ps_{b}", bufs=1, space="PSUM"))
        ms_ps = pp.tile([P, T], F32, name=f"ms_{b}")
        for half in range(2):
            ms = (2 * half, 2 * half + 1)
            ps = {
                m: pp.tile([P, T], F32, tag=f"o_ps{m - 2 * half}",
                           name=f"pso_{b}_{m}")
                for m in ms
            }
            for kp in range(NKP):
                atk = osp.tile([P, T], F16, tag="atk", bufs=4,
                               name=f"atk_{b}_{half}_{kp}")
                nc.sync.dma_start(out=atk[:], in_=att_rows(kp))
                wok = osp.tile([P, NH, P], F16, tag="wok", bufs=4,
                               name=f"wok_{b}_{half}_{kp}")
                nc.sync.dma_start(out=wok[:], in_=dp["wosh_t"][kp])
                for m in ms:
                    for ch in range(2):
                        nc.tensor.matmul(
                            ps[m][:, _cs(ch)],
                            wok[:, m, :],
                            atk[:, _cs(ch)],
                            start=(kp == 0),
                            stop=(kp == NKP - 1),
                        )
            for m in ms:
                xt = osp.tile([P, T], F16, tag="xsh", bufs=2, name=f"xs_{b}_{m}")
                nc.sync.dma_start(
                    out=xt[:], in_=dp["xsh_h"][m * P : (m + 1) * P, bs]
                )
                osb = osp.tile([P, T], F16, tag="osb", bufs=2, name=f"osb_{b}_{m}")
                nc.vector.tensor_add(osb[:], xt[:], ps[m][:])
                nc.gpsimd.dma_start(
                    out=dr[("xm_in", b)][m * P : (m + 1) * P, :], in_=osb[:]
                )
                sq = osp.tile([P, T], F16, tag="sq", bufs=2, name=f"sq_{b}_{m}")
                nc.scalar.activation(sq[:], osb[:], AF.Square)
                for ch in range(2):
                    nc.tensor.matmul(
                        ms_ps[:, _cs(ch)], cn["ones"][:], sq[:, _cs(ch)],
                        start=(m == 0), stop=(m == NH - 1),
                    )
        msb = osp.tile([P, T], F32, name=f"msb_{b}")
        nc.scalar.copy(msb[:], ms_ps[:])
        nc.gpsimd.dma_start(out=dr[("st_in", b)][:, :], in_=msb[:])
        nc.gpsimd.collective_compute(
            "AllGather", ALU.bypass, replica_groups=RG,
            ins=[dr[("xm_in", b)][:, :]], outs=[dr[("xm_all", b)][:, :]],
        )
        nc.gpsimd.collective_compute(
            "AllReduce", ALU.add, replica_groups=RG,
            ins=[dr[("st_in", b)][:, :]], outs=[dr[("st_out", b)][:, :]],
        )


def _mlp_begin(nc, tc, b, dp, cn, dr):
    """norm2 scale + x_mid tile loads with the m=0 ff (then up) matmuls
    run over the incoming stream — 2 PSUM banks so it can overlap the
    other batch's o-projection."""
    st = {"b": b}
    bph = ExitStack()
    st["stack"] = bph
    st["bp"] = bph.enter_context(tc.tile_pool(name=f"mlpb_{b}", bufs=1))
    st["sp"] = bph.enter_context(tc.tile_pool(name=f"mlp_{b}", bufs=1))
    ppre = tc.tile_pool(name=f"mlp_pre_ps_{b}", bufs=1, space="PSUM")
    st["ppre_cm"] = ppre
    ppre = ppre.__enter__()
    bp, sp = st["bp"], st["sp"]

    stb = sp.tile([P, T], F32, name=f"stb_{b}")
    nc.sync.dma_start(out=stb[:], in_=dr[("st_out", b)][:, :])
    lnt = sp.tile([P, T], F32, name=f"lnt_{b}")
    nc.scalar.activation(lnt[:], stb[:], AF.Ln, bias=cn["eps"][:], scale=1.0 / D)
    nc.scalar.activation(cn["bcast2"][b][:], lnt[:], AF.Exp, scale=-0.5)

    xmh = []
    st["xmh"] = xmh
    st["ffs"] = []
    st["hsb"] = []
    wt0 = {}
    for which, wsrc in (("f", dp["wf_t"]), ("u", dp["wu_t"])):
        wt = sp.tile([P, NKP, P], F16, tag="wffu", bufs=3, name=f"w{which}_{b}_0")
        nc.sync.dma_start(out=wt[:], in_=wsrc[0])
        wt0[which] = wt
    ps_f = ppre.tile([P, T], F32, tag="ps_pre", bufs=1, name=f"psf_{b}_0")
    for kp in range(NKP):
        xk = bp.tile([P, T], F16, tag=f"xm{kp}", name=f"xmh_{b}_{kp}")
        nc.sync.dma_start(out=xk[:], in_=dr[("xm_all", b)][kp * P : (kp + 1) * P, :])
        xmh.append(xk)
        for ch in range(2):
            nc.tensor.matmul(
                ps_f[:, _cs(ch)], wt0["f"][:, kp, :], xk[:, _cs(ch)],
                start=(kp == 0), stop=(kp == NKP - 1),
            )
    _ffu_evict(nc, st, cn, "f", 0, ps_f)
    ps_u = ppre.tile([P, T], F32, tag="ps_pre", bufs=1, name=f"psu_{b}_0")
    for kp in range(NKP):
        for ch in range(2):
            nc.tensor.matmul(
                ps_u[:, _cs(ch)], wt0["u"][:, kp, :], xmh[kp][:, _cs(ch)],
                start=(kp == 0), stop=(kp == NKP - 1),
            )
    _ffu_evict(nc, st, cn, "u", 0, ps_u)
    st["ppre_cm"].__exit__(None, None, None)
    return st


def _ffu_evict(nc, st, cn, which, m, ps):
    b, sp, bp = st["b"], st["sp"], st["bp"]
    nt = sp.tile([P, T], F16, tag=f"nrm_{which}", bufs=3, name=f"nt{which}_{b}_{m}")
    nc.vector.scalar_tensor_tensor(
        nt[:], ps[:], 1.0, cn["bcast2"][b][:], ALU.mult, ALU.mult,
    )
    if which == "f":
        ft = sp.tile([P, T], F16, tag="ffs", bufs=3, name=f"ff_{b}_{m}")
        nc.scalar.activation(ft[:], nt[:], AF.Silu)
        st["ffs"].append(ft)
    else:
        ht = bp.tile([P, T], F16, tag=f"h{m}", name=f"h_{b}_{m}")
        nc.vector.tensor_mul(ht[:], nt[:], st["ffs"][m][:])
        st["hsb"].append(ht)


def _mlp_finish(nc, tc, b, dp, cn, st):
    bs = slice(b * T, (b + 1) * T)
    sp, xmh, hsb = st["sp"], st["xmh"], st["hsb"]
    with st["stack"]:
        pp = st["stack"].enter_context(
            tc.tile_pool(name=f"mlp_ps_{b}", bufs=1, space="PSUM")
        )
        for m in range(1, NFT):
            for which, wsrc in (("f", dp["wf_t"]), ("u", dp["wu_t"])):
                wt = sp.tile([P, NKP, P], F16, tag="wffu", bufs=3,
                             name=f"w{which}_{b}_{m}")
                nc.sync.dma_start(out=wt[:], in_=wsrc[m])
                ps = pp.tile([P, T], F32, tag="ps_fu", bufs=2,
                             name=f"ps{which}_{b}_{m}")
                for kp in range(NKP):
                    for ch in range(2):
                        nc.tensor.matmul(
                            ps[:, _cs(ch)],
                            wt[:, kp, :],
                            xmh[kp][:, _cs(ch)],
                            start=(kp == 0),
                            stop=(kp == NKP - 1),
                        )
                _ffu_evict(nc, st, cn, which, m, ps)

        with ExitStack() as ph:
            wsp = ph.enter_context(tc.tile_pool(name=f"wo2_{b}", bufs=1))
            for dt in range(NDT):
                wt = wsp.tile([P, NFT, P], F16, tag="wot", bufs=3,
                              name=f"wot_{b}_{dt}")
                nc.sync.dma_start(out=wt[:], in_=dp["wout_t"][dt])
                ps = pp.tile([P, T], F32, tag="ps_fu", bufs=2,
                             name=f"pso2_{b}_{dt}")
                for m in range(NFT):
                    for ch in range(2):
                        nc.tensor.matmul(
                            ps[:, _cs(ch)],
                            wt[:, m, :],
                            hsb[m][:, _cs(ch)],
                            start=(m == 0),
                            stop=(m == NFT - 1),
                        )
                ysb = wsp.tile([P, T], F16, tag="ysb", bufs=3,
                               name=f"ysb_{b}_{dt}")
                nc.vector.scalar_tensor_tensor(
                    ysb[:], xmh[dt][:], 1.0 / N_CORES, ps[:],
                    ALU.mult, ALU.add,
                )
                nc.sync.dma_start(
                    out=dp["y"][dt * P : (dt + 1) * P, bs], in_=ysb[:]
                )


_NC_CACHE = {}


def _get_nc():
    if "nc" not in _NC_CACHE:
        _NC_CACHE["nc"] = _build()
    return _NC_CACHE["nc"]


def _host_prep(x, sin, cos, attn_norm_w, ff_norm_w, wq, wk, wv, wo, w_ff, w_up, w_out):
    f16 = np.float16
    x2 = np.asarray(x, np.float32).reshape(M, D)
    xT = np.ascontiguousarray(x2.T)
    rs1 = 1.0 / np.sqrt((xT * xT).mean(0) + EPS)        # [M] norm1 on host
    xnT = xT * rs1[None, :]

    sinT = np.asarray(sin, np.float32).reshape(M, HALF).T
    cosT = np.asarray(cos, np.float32).reshape(M, HALF).T
    cc = np.concatenate([cosT, cosT], axis=0)
    ss = np.concatenate([sinT, sinT], axis=0)
    css = np.stack([cc, ss]).astype(f16)

    anw = np.asarray(attn_norm_w, np.float32)[:, None]
    fnw = np.asarray(ff_norm_w, np.float32)[:, None]
    wqn = (anw * np.asarray(wq, np.float32)) * (H ** -0.5)
    wkn = anw * np.asarray(wk, np.float32)
    wvn = anw * np.asarray(wv, np.float32)
    wfn = fnw * np.asarray(w_ff, np.float32)
    wun = fnw * np.asarray(w_up, np.float32)
    wo = np.asarray(wo, np.float32)
    w_out = np.asarray(w_out, np.float32)
    xT_h = xT.astype(f16)

    def mtile(w):
        # [K, F] -> [F/P, P, K/P, P] with [m, p, kp, j] = w[kp*P+p, m*P+j]
        K, F = w.shape
        return np.ascontiguousarray(
            w.reshape(K // P, P, F // P, P).transpose(2, 1, 0, 3)
        )

    in_maps = []
    for c in range(N_CORES):
        qs = slice(c * QC, (c + 1) * QC)
        fs = slice(c * FC, (c + 1) * FC)
        in_maps.append(
            {
                "xnT_h": xnT.astype(f16),
                "xsh_h": xT_h[qs, :],
                "css": css,
                "wq_t": mtile(wqn[:, qs]).astype(f16),
                "wk_t": mtile(wkn[:, qs]).astype(f16),
                "wv_n": wvn[:, qs].astype(f16),
                # [kp, p, m, j] = wo[kp*P+p, c*QC + m*P + j]
                "wosh_t": np.ascontiguousarray(
                    wo[:, qs].reshape(NKP, P, NH, P)
                ).astype(f16),
                "wf_t": mtile(wfn[:, fs]).astype(f16),
                "wu_t": mtile(wun[:, fs]).astype(f16),
                "wout_t": mtile(w_out[fs, :]).astype(f16),
            }
        )
    return in_maps


def kernel(**inputs) -> np.ndarray:
    nc = _get_nc()
    in_maps = _host_prep(**inputs)
    res = run_bass_kernel_spmd(
        nc, in_maps, core_ids=list(range(N_CORES)), trace=False
    )
    acc = res.results[0]["y"].astype(np.float64)
    for c in range(1, N_CORES):
        acc += res.results[c]["y"]
    return np.ascontiguousarray(acc.T).astype(np.float32).reshape(B, T, D)
